# revision 20
# baseline (speedup 1.0000x reference)
"""Trainium2 Bass kernel for nn_CrowdsClassificationCModel.

Computes, for B x (C,C,R) annotator confusion tensors:
    logits = einsum('bf,fkr->bkr', x, W).reshape(B,C,C,R) + b
    M      = softmax(logits, axis=2)           # over predicted-class d
    out    = einsum('bc,bcdr->bdr', p, M)      # (B, C, R)

Sharding: pure data-parallel over B across 8 NeuronCores; W/b replicated.

Per-core dataflow (Bs = 2048 batch rows; k = c*512 + d*64 + r, 32 chunks of 128):
  - PE:  logits chunks (128k x 512b) via fp8e4m3 DoubleRow matmuls.
         The f=128 contraction is packed as 2 k-tiles of 64 partitions plus
         a 65th constant-one row that carries the bias b (so softmax bias
         needs no separate add anywhere). x, W scaled by 16 on host; the
         exp activation descales by 1/256.
  - ACT: E = exp(logits/256)  PSUM->SBUF bf16, 1536-wide instructions,
         into one big per-chunk E tile (128 x 32 x 512)
  - PE:  S_dup (128 x 512b) = sum_d E via exact 0/1 eye64 masked matmuls
  - DVE: sinv ~= 1/S;  Pool: qd = sinv * p_bcast
  - DVE/Pool: Eq = E * qd in place (bf16 TT, one 4-slice op per c, engines
         split by c parity)
  - PE:  c-sum for output groups 0..2: psum += I.T @ Eq (exact identity),
         interleaved one group per prev chunk c-step; DVE copies PSUM->SBUF
  - DVE: c-sum for output group 3 as incremental bf16 adds (no PSUM)
  - DMA out from SBUF (k-major; host transposes after gather)
"""

import numpy as np
import ml_dtypes

BF = ml_dtypes.bfloat16
F8 = ml_dtypes.float8_e4m3
NCORES = 8
B_FULL = 16384
BS = B_FULL // NCORES   # 2048 rows per core
F = 128
C = 8
R = 64
K = C * C * R           # 4096
NCHUNK = K // 128       # 32 k-chunks
NB = 4                  # b-chunks of 512 per core
BCH = BS // NB          # 512
FP8_SCALE = 16.0        # x and W each scaled by 16 -> exp descales by 1/256
PSUM_J = (2,) * 16      # k-chunks per logits psum tile

_CACHE = {}


def _build_nc():
    import concourse.bass as bass
    import concourse.bacc as bacc
    import concourse.tile as tile
    from concourse import mybir
    from contextlib import ExitStack

    f32 = mybir.dt.float32
    bf16 = mybir.dt.bfloat16
    fp8 = mybir.dt.float8e4
    Exp = mybir.ActivationFunctionType.Exp
    MUL = mybir.AluOpType.mult
    ADD = mybir.AluOpType.add
    DR = mybir.MatmulPerfMode.DoubleRow

    nc = bacc.Bacc()
    x8 = nc.declare_dram_parameter("x8", [65, 2 * BS], fp8, isOutput=False)
    W8 = nc.declare_dram_parameter("W8", [65, 2 * K], fp8, isOutput=False)
    pT = nc.declare_dram_parameter("pT", [C, BS], bf16, isOutput=False)
    eyes = nc.declare_dram_parameter("eyes", [128, 256], bf16, isOutput=False)
    # k-major output: row k' = d*64+r, col b; host transposes after gather
    out = nc.declare_dram_parameter("out", [C * R, BS], f32, isOutput=True)

    with ExitStack() as ctx:
        tc = ctx.enter_context(tile.TileContext(nc))
        const = ctx.enter_context(tc.tile_pool(name="const", bufs=1))
        epool = ctx.enter_context(tc.tile_pool(name="e", bufs=2))
        sm = ctx.enter_context(tc.tile_pool(name="sm", bufs=3))
        pbp = ctx.enter_context(tc.tile_pool(name="pbp", bufs=2))
        plg = ctx.enter_context(tc.tile_pool(name="plg", bufs=2, space="PSUM"))
        ps = ctx.enter_context(tc.tile_pool(name="ps", bufs=2, space="PSUM"))
        pot = ctx.enter_context(tc.tile_pool(name="pot", bufs=2, space="PSUM"))

        # const loads: W8 pieces first on SP (PE needs chunk 0 first), x8 on
        # the Pool queue, eye/identity follow on SP before first use
        W8s = const.tile([65, 2, K], fp8)
        x8s = const.tile([65, 2, BS], fp8)
        nc.sync.dma_start(out=W8s[:, :, 0:512], in_=W8[:, 0:1024])
        for i in range(NB):
            nc.gpsimd.dma_start(out=x8s[:, :, i * BCH:(i + 1) * BCH],
                                in_=x8[:, i * 2 * BCH:(i + 1) * 2 * BCH])
        for i in range(1, 8):
            nc.sync.dma_start(out=W8s[:, :, i * 512:(i + 1) * 512],
                              in_=W8[:, i * 1024:(i + 1) * 1024])
        eye64s = const.tile([128, 128], bf16)   # eye64 dup mask (S-sum)
        nc.sync.dma_start(out=eye64s, in_=eyes[:, 0:128])
        id128s = const.tile([128, 128], bf16)   # identity (out c-sum)
        nc.sync.dma_start(out=id128s, in_=eyes[:, 128:256])
        # warm the Exp activation table while DMAs stream in
        warm = const.tile([128, 1], f32)
        nc.gpsimd.memset(warm, 0.0)
        nc.scalar.activation(out=warm, in_=warm, func=Exp)

        def emit_out_group_pe(bigE, bc, g):
            """c-sum for output k-chunk g (0..2) of batch-chunk bc."""
            pog = pot.tile([128, BCH], f32, tag="ot")
            for c in range(C):
                nc.tensor.matmul(
                    pog, lhsT=id128s, rhs=bigE[:, c * 4 + g, :],
                    start=(c == 0), stop=(c == C - 1),
                )
            osb = sm.tile([128, BCH], f32, tag="osb")
            nc.vector.tensor_copy(out=osb, in_=pog)
            nc.gpsimd.dma_start(
                out=out[g * 128:(g + 1) * 128, bc * BCH:(bc + 1) * BCH],
                in_=osb,
            )

        def load_pb(bc):
            # batched broadcast load of p for all 8 classes of b-chunk bc:
            # pb_all[pp, c*512 + i] = p[bc*512 + i, c]  (same for all pp)
            t = pbp.tile([128, C, BCH], bf16, tag="pb")
            nc.sync.dma_start(
                out=t,
                in_=bass.AP(tensor=pT.handle if hasattr(pT, "handle") else pT,
                            offset=bc * BCH,
                            ap=[[0, 128], [BS, C], [1, BCH]]),
            )
            return t

        EMIT_AT = {1: 0, 4: 1, 6: 2}   # prev-chunk PE emissions: c-step -> g
        pb_next = load_pb(0)
        prev = None  # (bc, bigE) awaiting PE-group emission
        for bc in range(NB):
            bsl = slice(bc * BCH, (bc + 1) * BCH)
            pb_all = pb_next
            bigE = epool.tile([128, NCHUNK, BCH], bf16, tag="bigE")
            # groups accumulated as elementwise adds instead of PE matmuls:
            # always g3 (DVE); on the last chunk also g0/g1 (DVE) and g2
            # (Pool) so the kernel tail has no serial PE emission phase
            last = bc == NB - 1
            adds = ({0: nc.vector, 1: nc.vector, 2: nc.gpsimd, 3: nc.vector}
                    if last else {3: nc.vector})
            accs = {g: sm.tile([128, BCH], bf16, tag=f"acc{g}",
                               name=f"acc{g}_{bc}") for g in adds}
            j0 = 0
            next_c = 0

            def process_c(c):
                nonlocal pb_next
                # weighted d-sum -> S duplicated over both 64-partition halves
                sps = ps.tile([128, BCH], f32, tag="s")
                for dj in range(4):
                    nc.tensor.matmul(
                        sps, lhsT=eye64s, rhs=bigE[:, c * 4 + dj, :],
                        start=(dj == 0), stop=(dj == 3),
                    )
                sinv = sm.tile([128, BCH], f32, tag="sinv")
                nc.vector.reciprocal_approx_fast(out=sinv, in_=sps)
                qd = sm.tile([128, BCH], bf16, tag="qd")
                nc.gpsimd.tensor_tensor(out=qd, in0=sinv, in1=pb_all[:, c, :],
                                        op=MUL)
                # Eq = E * qd in place over all 4 slices of c in one op
                qd_b = bass.AP(tensor=qd.tensor, offset=qd.offset,
                               ap=[list(qd.ap[0]), [0, 4], [1, BCH]])
                eng = nc.vector if c % 2 == 0 else nc.gpsimd
                eng.tensor_tensor(
                    out=bigE[:, c * 4:(c + 1) * 4, :],
                    in0=bigE[:, c * 4:(c + 1) * 4, :],
                    in1=qd_b, op=MUL)
                # add-accumulated output groups: incremental, no PSUM
                for g, eng2 in adds.items():
                    if c == 1:
                        eng2.tensor_tensor(out=accs[g], in0=bigE[:, 0 * 4 + g, :],
                                           in1=bigE[:, 1 * 4 + g, :], op=ADD)
                    elif 2 <= c < C - 1:
                        eng2.tensor_tensor(out=accs[g], in0=accs[g],
                                           in1=bigE[:, c * 4 + g, :], op=ADD)
                    elif c == C - 1:
                        osbg = sm.tile([128, BCH], f32, tag=f"osbg{g}")
                        eng2.tensor_tensor(out=osbg, in0=accs[g],
                                           in1=bigE[:, c * 4 + g, :], op=ADD)
                        nc.gpsimd.dma_start(
                            out=out[g * 128:(g + 1) * 128, bsl], in_=osbg)
                # interleave previous chunk's output phase to keep PE dense
                if prev is not None and c in EMIT_AT:
                    emit_out_group_pe(prev[1], prev[0], EMIT_AT[c])
                # prefetch next b-chunk's p broadcast mid-chunk
                if c == 3 and bc + 1 < NB:
                    pb_next = load_pb(bc + 1)

            for gsz in PSUM_J:
                lg = plg.tile([128, 2, BCH], f32, tag="lg")
                for t in range(gsz):
                    j = j0 + t
                    nc.tensor.matmul(
                        lg[:, t, :],
                        lhsT=W8s[:, :, j * 128:(j + 1) * 128],
                        rhs=x8s[:, :, bsl],
                        start=True, stop=True,
                        perf_mode=DR,
                    )
                nc.scalar.activation(
                    out=bigE[:, j0:j0 + gsz, :], in_=lg[:, 0:gsz, :], func=Exp,
                    scale=1.0 / (FP8_SCALE * FP8_SCALE))
                j0 += gsz
                while next_c < C and next_c * 4 + 3 < j0:
                    process_c(next_c)
                    next_c += 1
            prev = (bc, bigE)
    nc.compile()
    return nc


def _host_prep(x, p, W, b):
    W2 = W.reshape(F, K)
    bflat = b.reshape(K).astype(np.float32)
    # fp8 DoubleRow packing: [65, 2, *]; f split into two 64-deep k-tiles,
    # 65th row carries (1, bias) so the matmul adds b before exp
    S2 = FP8_SCALE * FP8_SCALE
    x8_all = np.zeros((65, 2, B_FULL), dtype=F8)
    xT = (x.T * FP8_SCALE).astype(F8)                      # (128, B)
    x8_all[:64, 0] = xT[:64]
    x8_all[:64, 1] = xT[64:]
    x8_all[64, 0] = np.float32(1.0)
    W8v = np.zeros((65, 2, K), dtype=F8)
    W8q = (W2 * FP8_SCALE).astype(F8)                      # (128, K)
    W8v[:64, 0] = W8q[:64]
    W8v[:64, 1] = W8q[64:]
    W8v[64, 0] = (bflat * S2).astype(F8)
    # pack to match the 8-piece on-chip loads: piece i holds [t=0 | t=1] for
    # k-block i*512:(i+1)*512 (dest tile iterates (t, k) per piece)
    W8 = np.ascontiguousarray(
        W8v.reshape(65, 2, 8, 512).transpose(0, 2, 1, 3)
    ).reshape(65, 2 * K)
    eye64 = ((np.arange(128)[:, None] % 64) == (np.arange(128)[None, :] % 64))
    eyes = np.zeros((128, 256), dtype=BF)
    eyes[:, 0:128] = eye64.astype(BF)
    eyes[:, 128:256] = np.eye(128, dtype=np.float32).astype(BF)
    pT_all = np.ascontiguousarray(p.T.astype(BF))          # (8, B)
    in_maps = []
    for ci in range(NCORES):
        sl = slice(ci * BS, (ci + 1) * BS)
        # piece-major x8: piece i holds [t=0 | t=1] for b-block i*512:(i+1)*512
        x8c = np.ascontiguousarray(
            x8_all[:, :, sl].reshape(65, 2, NB, BCH).transpose(0, 2, 1, 3)
        ).reshape(65, 2 * BS)
        in_maps.append({
            "x8": x8c,
            "W8": W8,
            "pT": np.ascontiguousarray(pT_all[:, sl]),
            "eyes": eyes,
        })
    return in_maps


def kernel(x, p, W, b):
    from concourse.bass_utils import run_bass_kernel_spmd

    if "nc" not in _CACHE:
        _CACHE["nc"] = _build_nc()
    nc = _CACHE["nc"]
    in_maps = _host_prep(np.asarray(x), np.asarray(p), np.asarray(W), np.asarray(b))
    res = run_bass_kernel_spmd(nc, in_maps, list(range(NCORES)))
    outs = [np.asarray(res.results[i]["out"]) for i in range(NCORES)]  # (C*R, BS)
    full = np.concatenate(outs, axis=1)              # (C*R, B)
    full = np.ascontiguousarray(full.T)              # (B, C*R)
    return full.reshape(B_FULL, C, R).astype(np.float32)


# revision 21
# speedup vs baseline: 1.0203x; 1.0203x over previous
"""Trainium2 Bass kernel for nn_CrowdsClassificationCModel.

Computes, for B x (C,C,R) annotator confusion tensors:
    logits = einsum('bf,fkr->bkr', x, W).reshape(B,C,C,R) + b
    M      = softmax(logits, axis=2)           # over predicted-class d
    out    = einsum('bc,bcdr->bdr', p, M)      # (B, C, R)

Sharding: pure data-parallel over B across 8 NeuronCores; W/b replicated.

Per-core dataflow (Bs = 2048 batch rows; k = c*512 + d*64 + r, 32 chunks of 128):
  - PE:  logits chunks (128k x 512b) via fp8e4m3 DoubleRow matmuls.
         The f=128 contraction is packed as 2 k-tiles of 64 partitions plus
         a 65th constant-one row that carries the bias b (so softmax bias
         needs no separate add anywhere). x, W scaled by 16 on host; the
         exp activation descales by 1/256.
  - ACT: E = exp(logits/256)  PSUM->SBUF bf16, 1536-wide instructions,
         into one big per-chunk E tile (128 x 32 x 512)
  - PE:  S_dup (128 x 512b) = sum_d E via exact 0/1 eye64 masked matmuls
  - DVE: sinv ~= 1/S;  Pool: qd = sinv * p_bcast
  - DVE/Pool: Eq = E * qd in place (bf16 TT, one 4-slice op per c, engines
         split by c parity)
  - PE:  c-sum for output groups 0..2: psum += I.T @ Eq (exact identity),
         interleaved one group per prev chunk c-step; DVE copies PSUM->SBUF
  - DVE: c-sum for output group 3 as incremental bf16 adds (no PSUM)
  - DMA out from SBUF (k-major; host transposes after gather)
"""

import numpy as np
import ml_dtypes

BF = ml_dtypes.bfloat16
F8 = ml_dtypes.float8_e4m3
NCORES = 8
B_FULL = 16384
BS = B_FULL // NCORES   # 2048 rows per core
F = 128
C = 8
R = 64
K = C * C * R           # 4096
NCHUNK = K // 128       # 32 k-chunks
NB = 4                  # b-chunks of 512 per core
BCH = BS // NB          # 512
FP8_SCALE = 16.0        # x and W each scaled by 16 -> exp descales by 1/256
PSUM_J = (2,) * 16      # k-chunks per logits psum tile

_CACHE = {}


def _build_nc():
    import concourse.bass as bass
    import concourse.bacc as bacc
    import concourse.tile as tile
    from concourse import mybir
    from contextlib import ExitStack

    f32 = mybir.dt.float32
    bf16 = mybir.dt.bfloat16
    fp8 = mybir.dt.float8e4
    Exp = mybir.ActivationFunctionType.Exp
    MUL = mybir.AluOpType.mult
    ADD = mybir.AluOpType.add
    DR = mybir.MatmulPerfMode.DoubleRow

    nc = bacc.Bacc()
    x8 = nc.declare_dram_parameter("x8", [65, 2 * BS], fp8, isOutput=False)
    W8 = nc.declare_dram_parameter("W8", [65, 2 * K], fp8, isOutput=False)
    pT = nc.declare_dram_parameter("pT", [C, BS], bf16, isOutput=False)
    eyes = nc.declare_dram_parameter("eyes", [128, 256], bf16, isOutput=False)
    # k-major output: row k' = d*64+r, col b; host transposes after gather
    out = nc.declare_dram_parameter("out", [C * R, BS], f32, isOutput=True)

    with ExitStack() as ctx:
        tc = ctx.enter_context(tile.TileContext(nc))
        const = ctx.enter_context(tc.tile_pool(name="const", bufs=1))
        epool = ctx.enter_context(tc.tile_pool(name="e", bufs=2))
        sm = ctx.enter_context(tc.tile_pool(name="sm", bufs=3))
        pbp = ctx.enter_context(tc.tile_pool(name="pbp", bufs=2))
        plg = ctx.enter_context(tc.tile_pool(name="plg", bufs=2, space="PSUM"))
        ps = ctx.enter_context(tc.tile_pool(name="ps", bufs=2, space="PSUM"))
        pot = ctx.enter_context(tc.tile_pool(name="pot", bufs=2, space="PSUM"))

        # const loads: W8 pieces first on SP (PE needs chunk 0 first), x8 on
        # the Pool queue, eye/identity follow on SP before first use
        W8s = const.tile([65, 2, K], fp8)
        x8s = const.tile([65, 2, BS], fp8)
        nc.sync.dma_start(out=W8s[:, :, 0:512], in_=W8[:, 0:1024])
        for i in range(NB):
            nc.gpsimd.dma_start(out=x8s[:, :, i * BCH:(i + 1) * BCH],
                                in_=x8[:, i * 2 * BCH:(i + 1) * 2 * BCH])
        for i in range(1, 8):
            nc.sync.dma_start(out=W8s[:, :, i * 512:(i + 1) * 512],
                              in_=W8[:, i * 1024:(i + 1) * 1024])
        eye64s = const.tile([128, 128], bf16)   # eye64 dup mask (S-sum)
        nc.sync.dma_start(out=eye64s, in_=eyes[:, 0:128])
        id128s = const.tile([128, 128], bf16)   # identity (out c-sum)
        nc.sync.dma_start(out=id128s, in_=eyes[:, 128:256])
        # warm the Exp activation table while DMAs stream in
        warm = const.tile([128, 1], f32)
        nc.gpsimd.memset(warm, 0.0)
        nc.scalar.activation(out=warm, in_=warm, func=Exp)

        def emit_out_group_pe(bigE, bc, g):
            """c-sum for output k-chunk g (0..2) of batch-chunk bc."""
            pog = pot.tile([128, BCH], f32, tag="ot")
            for c in range(C):
                nc.tensor.matmul(
                    pog, lhsT=id128s, rhs=bigE[:, c * 4 + g, :],
                    start=(c == 0), stop=(c == C - 1),
                )
            osb = sm.tile([128, BCH], f32, tag="osb")
            nc.vector.tensor_copy(out=osb, in_=pog)
            nc.sync.dma_start(
                out=out[g * 128:(g + 1) * 128, bc * BCH:(bc + 1) * BCH],
                in_=osb,
            )

        def load_pb(bc):
            # batched broadcast load of p for all 8 classes of b-chunk bc:
            # pb_all[pp, c*512 + i] = p[bc*512 + i, c]  (same for all pp)
            t = pbp.tile([128, C, BCH], bf16, tag="pb")
            nc.sync.dma_start(
                out=t,
                in_=bass.AP(tensor=pT.handle if hasattr(pT, "handle") else pT,
                            offset=bc * BCH,
                            ap=[[0, 128], [BS, C], [1, BCH]]),
            )
            return t

        EMIT_AT = {1: 0, 4: 1, 6: 2}   # prev-chunk PE emissions: c-step -> g
        pb_next = load_pb(0)
        prev = None  # (bc, bigE) awaiting PE-group emission
        for bc in range(NB):
            bsl = slice(bc * BCH, (bc + 1) * BCH)
            pb_all = pb_next
            bigE = epool.tile([128, NCHUNK, BCH], bf16, tag="bigE")
            # groups accumulated as elementwise adds instead of PE matmuls:
            # always g3 (DVE); on the last chunk also g0/g1 (DVE) and g2
            # (Pool) so the kernel tail has no serial PE emission phase
            last = bc == NB - 1
            adds = ({0: nc.vector, 1: nc.vector, 2: nc.gpsimd, 3: nc.vector}
                    if last else {3: nc.vector})
            accs = {g: sm.tile([128, BCH], bf16, tag=f"acc{g}",
                               name=f"acc{g}_{bc}") for g in adds}
            j0 = 0
            next_c = 0

            def process_c(c):
                nonlocal pb_next
                # weighted d-sum -> S duplicated over both 64-partition halves
                sps = ps.tile([128, BCH], f32, tag="s")
                for dj in range(4):
                    nc.tensor.matmul(
                        sps, lhsT=eye64s, rhs=bigE[:, c * 4 + dj, :],
                        start=(dj == 0), stop=(dj == 3),
                    )
                sinv = sm.tile([128, BCH], f32, tag="sinv")
                nc.vector.reciprocal_approx_fast(out=sinv, in_=sps)
                qd = sm.tile([128, BCH], bf16, tag="qd")
                nc.gpsimd.tensor_tensor(out=qd, in0=sinv, in1=pb_all[:, c, :],
                                        op=MUL)
                # Eq = E * qd in place over all 4 slices of c in one op
                qd_b = bass.AP(tensor=qd.tensor, offset=qd.offset,
                               ap=[list(qd.ap[0]), [0, 4], [1, BCH]])
                eng = nc.vector if c % 2 == 0 else nc.gpsimd
                eng.tensor_tensor(
                    out=bigE[:, c * 4:(c + 1) * 4, :],
                    in0=bigE[:, c * 4:(c + 1) * 4, :],
                    in1=qd_b, op=MUL)
                # add-accumulated output groups: incremental, no PSUM
                for g, eng2 in adds.items():
                    if c == 1:
                        eng2.tensor_tensor(out=accs[g], in0=bigE[:, 0 * 4 + g, :],
                                           in1=bigE[:, 1 * 4 + g, :], op=ADD)
                    elif 2 <= c < C - 1:
                        eng2.tensor_tensor(out=accs[g], in0=accs[g],
                                           in1=bigE[:, c * 4 + g, :], op=ADD)
                    elif c == C - 1:
                        osbg = sm.tile([128, BCH], f32, tag=f"osbg{g}")
                        eng2.tensor_tensor(out=osbg, in0=accs[g],
                                           in1=bigE[:, c * 4 + g, :], op=ADD)
                        nc.sync.dma_start(
                            out=out[g * 128:(g + 1) * 128, bsl], in_=osbg)
                # interleave previous chunk's output phase to keep PE dense
                if prev is not None and c in EMIT_AT:
                    emit_out_group_pe(prev[1], prev[0], EMIT_AT[c])
                # prefetch next b-chunk's p broadcast mid-chunk
                if c == 3 and bc + 1 < NB:
                    pb_next = load_pb(bc + 1)

            for gsz in PSUM_J:
                lg = plg.tile([128, 2, BCH], f32, tag="lg")
                for t in range(gsz):
                    j = j0 + t
                    nc.tensor.matmul(
                        lg[:, t, :],
                        lhsT=W8s[:, :, j * 128:(j + 1) * 128],
                        rhs=x8s[:, :, bsl],
                        start=True, stop=True,
                        perf_mode=DR,
                    )
                nc.scalar.activation(
                    out=bigE[:, j0:j0 + gsz, :], in_=lg[:, 0:gsz, :], func=Exp,
                    scale=1.0 / (FP8_SCALE * FP8_SCALE))
                j0 += gsz
                while next_c < C and next_c * 4 + 3 < j0:
                    process_c(next_c)
                    next_c += 1
            prev = (bc, bigE)
    nc.compile()
    return nc


def _host_prep(x, p, W, b):
    W2 = W.reshape(F, K)
    bflat = b.reshape(K).astype(np.float32)
    # fp8 DoubleRow packing: [65, 2, *]; f split into two 64-deep k-tiles,
    # 65th row carries (1, bias) so the matmul adds b before exp
    S2 = FP8_SCALE * FP8_SCALE
    x8_all = np.zeros((65, 2, B_FULL), dtype=F8)
    xT = (x.T * FP8_SCALE).astype(F8)                      # (128, B)
    x8_all[:64, 0] = xT[:64]
    x8_all[:64, 1] = xT[64:]
    x8_all[64, 0] = np.float32(1.0)
    W8v = np.zeros((65, 2, K), dtype=F8)
    W8q = (W2 * FP8_SCALE).astype(F8)                      # (128, K)
    W8v[:64, 0] = W8q[:64]
    W8v[:64, 1] = W8q[64:]
    W8v[64, 0] = (bflat * S2).astype(F8)
    # pack to match the 8-piece on-chip loads: piece i holds [t=0 | t=1] for
    # k-block i*512:(i+1)*512 (dest tile iterates (t, k) per piece)
    W8 = np.ascontiguousarray(
        W8v.reshape(65, 2, 8, 512).transpose(0, 2, 1, 3)
    ).reshape(65, 2 * K)
    eye64 = ((np.arange(128)[:, None] % 64) == (np.arange(128)[None, :] % 64))
    eyes = np.zeros((128, 256), dtype=BF)
    eyes[:, 0:128] = eye64.astype(BF)
    eyes[:, 128:256] = np.eye(128, dtype=np.float32).astype(BF)
    pT_all = np.ascontiguousarray(p.T.astype(BF))          # (8, B)
    in_maps = []
    for ci in range(NCORES):
        sl = slice(ci * BS, (ci + 1) * BS)
        # piece-major x8: piece i holds [t=0 | t=1] for b-block i*512:(i+1)*512
        x8c = np.ascontiguousarray(
            x8_all[:, :, sl].reshape(65, 2, NB, BCH).transpose(0, 2, 1, 3)
        ).reshape(65, 2 * BS)
        in_maps.append({
            "x8": x8c,
            "W8": W8,
            "pT": np.ascontiguousarray(pT_all[:, sl]),
            "eyes": eyes,
        })
    return in_maps


def kernel(x, p, W, b):
    from concourse.bass_utils import run_bass_kernel_spmd

    if "nc" not in _CACHE:
        _CACHE["nc"] = _build_nc()
    nc = _CACHE["nc"]
    in_maps = _host_prep(np.asarray(x), np.asarray(p), np.asarray(W), np.asarray(b))
    res = run_bass_kernel_spmd(nc, in_maps, list(range(NCORES)))
    outs = [np.asarray(res.results[i]["out"]) for i in range(NCORES)]  # (C*R, BS)
    full = np.concatenate(outs, axis=1)              # (C*R, B)
    full = np.ascontiguousarray(full.T)              # (B, C*R)
    return full.reshape(B_FULL, C, R).astype(np.float32)


# revision 22
# speedup vs baseline: 1.0317x; 1.0112x over previous
"""Trainium2 Bass kernel for nn_CrowdsClassificationCModel.

Computes, for B x (C,C,R) annotator confusion tensors:
    logits = einsum('bf,fkr->bkr', x, W).reshape(B,C,C,R) + b
    M      = softmax(logits, axis=2)           # over predicted-class d
    out    = einsum('bc,bcdr->bdr', p, M)      # (B, C, R)

Sharding: pure data-parallel over B across 8 NeuronCores; W/b replicated.

Per-core dataflow (Bs = 2048 batch rows; k = c*512 + d*64 + r, 32 chunks of 128):
  - PE:  logits chunks (128k x 512b) via fp8e4m3 DoubleRow matmuls.
         The f=128 contraction is packed as 2 k-tiles of 64 partitions plus
         a 65th constant-one row that carries the bias b (so softmax bias
         needs no separate add anywhere). x, W scaled by 16 on host; the
         exp activation descales by 1/256.
  - ACT: E = exp(logits/256)  PSUM->SBUF bf16, 1536-wide instructions,
         into one big per-chunk E tile (128 x 32 x 512)
  - PE:  S_dup (128 x 512b) = sum_d E via exact 0/1 eye64 masked matmuls
  - DVE: sinv ~= 1/S;  Pool: qd = sinv * p_bcast
  - DVE/Pool: Eq = E * qd in place (bf16 TT, one 4-slice op per c, engines
         split by c parity)
  - PE:  c-sum for output groups 0..2: psum += I.T @ Eq (exact identity),
         interleaved one group per prev chunk c-step; DVE copies PSUM->SBUF
  - DVE: c-sum for output group 3 as incremental bf16 adds (no PSUM)
  - DMA out from SBUF (k-major; host transposes after gather)
"""

import numpy as np
import ml_dtypes

BF = ml_dtypes.bfloat16
F8 = ml_dtypes.float8_e4m3
NCORES = 8
B_FULL = 16384
BS = B_FULL // NCORES   # 2048 rows per core
F = 128
C = 8
R = 64
K = C * C * R           # 4096
NCHUNK = K // 128       # 32 k-chunks
NB = 4                  # b-chunks of 512 per core
BCH = BS // NB          # 512
FP8_SCALE = 16.0        # x and W each scaled by 16 -> exp descales by 1/256
PSUM_J = (2,) * 16      # k-chunks per logits psum tile

_CACHE = {}


def _build_nc():
    import concourse.bass as bass
    import concourse.bacc as bacc
    import concourse.tile as tile
    from concourse import mybir
    from contextlib import ExitStack

    f32 = mybir.dt.float32
    bf16 = mybir.dt.bfloat16
    fp8 = mybir.dt.float8e4
    Exp = mybir.ActivationFunctionType.Exp
    MUL = mybir.AluOpType.mult
    ADD = mybir.AluOpType.add
    DR = mybir.MatmulPerfMode.DoubleRow

    nc = bacc.Bacc()
    x8 = nc.declare_dram_parameter("x8", [65, 2 * BS], fp8, isOutput=False)
    W8 = nc.declare_dram_parameter("W8", [65, 2 * K], fp8, isOutput=False)
    pT = nc.declare_dram_parameter("pT", [C, BS], bf16, isOutput=False)
    eyes = nc.declare_dram_parameter("eyes", [128, 256], bf16, isOutput=False)
    # k-major output: row k' = d*64+r, col b; host transposes after gather
    out = nc.declare_dram_parameter("out", [C * R, BS], f32, isOutput=True)

    with ExitStack() as ctx:
        tc = ctx.enter_context(tile.TileContext(nc))
        const = ctx.enter_context(tc.tile_pool(name="const", bufs=1))
        epool = ctx.enter_context(tc.tile_pool(name="e", bufs=2))
        sm = ctx.enter_context(tc.tile_pool(name="sm", bufs=3))
        pbp = ctx.enter_context(tc.tile_pool(name="pbp", bufs=2))
        plg = ctx.enter_context(tc.tile_pool(name="plg", bufs=2, space="PSUM"))
        ps = ctx.enter_context(tc.tile_pool(name="ps", bufs=2, space="PSUM"))
        pot = ctx.enter_context(tc.tile_pool(name="pot", bufs=2, space="PSUM"))

        # const loads: W8 pieces first on SP (PE needs chunk 0 first), x8 on
        # the Pool queue, eye/identity follow on SP before first use
        W8s = const.tile([65, 2, K], fp8)
        x8s = const.tile([65, 2, BS], fp8)
        nc.sync.dma_start(out=W8s[:, :, 0:512], in_=W8[:, 0:1024])
        for i in range(NB):
            nc.gpsimd.dma_start(out=x8s[:, :, i * BCH:(i + 1) * BCH],
                                in_=x8[:, i * 2 * BCH:(i + 1) * 2 * BCH])
        eye64s = const.tile([128, 128], bf16)   # eye64 dup mask (S-sum)
        nc.sync.dma_start(out=eye64s, in_=eyes[:, 0:128])
        id128s = const.tile([128, 128], bf16)   # identity (out c-sum)
        nc.sync.dma_start(out=id128s, in_=eyes[:, 128:256])
        for i in range(1, 4):
            nc.sync.dma_start(out=W8s[:, :, i * 512:(i + 1) * 512],
                              in_=W8[:, i * 1024:(i + 1) * 1024])
        # warm the Exp activation table while DMAs stream in
        warm = const.tile([128, 1], f32)
        nc.gpsimd.memset(warm, 0.0)
        nc.scalar.activation(out=warm, in_=warm, func=Exp)

        def emit_out_group_pe(bigE, bc, g):
            """c-sum for output k-chunk g (0..2) of batch-chunk bc."""
            pog = pot.tile([128, BCH], f32, tag="ot")
            for c in range(C):
                nc.tensor.matmul(
                    pog, lhsT=id128s, rhs=bigE[:, c * 4 + g, :],
                    start=(c == 0), stop=(c == C - 1),
                )
            osb = sm.tile([128, BCH], f32, tag="osb")
            if last_chunk_copy[0]:
                nc.scalar.copy(out=osb, in_=pog)
            else:
                nc.vector.tensor_copy(out=osb, in_=pog)
            nc.sync.dma_start(
                out=out[g * 128:(g + 1) * 128, bc * BCH:(bc + 1) * BCH],
                in_=osb,
            )

        def load_pb(bc, split=False):
            # batched broadcast load of p for all 8 classes of b-chunk bc:
            # pb_all[pp, c*512 + i] = p[bc*512 + i, c]  (same for all pp)
            t = pbp.tile([128, C, BCH], bf16, tag="pb")
            pTh = pT.handle if hasattr(pT, "handle") else pT
            # split=True loads per-c pieces so c=0's slice lands early
            for c0, cn in ([(0, 1), (1, 1), (2, 2), (4, 4)] if split
                           else [(0, C)]):
                nc.sync.dma_start(
                    out=t[:, c0:c0 + cn, :],
                    in_=bass.AP(tensor=pTh, offset=bc * BCH + c0 * BS,
                                ap=[[0, 128], [BS, cn], [1, BCH]]),
                )
            return t

        EMIT_AT = {1: 0, 4: 1, 6: 2}   # prev-chunk PE emissions: c-step -> g
        last_chunk_copy = [False]
        pb_next = load_pb(0, split=True)
        for i in range(4, 8):
            nc.sync.dma_start(out=W8s[:, :, i * 512:(i + 1) * 512],
                              in_=W8[:, i * 1024:(i + 1) * 1024])
        prev = None  # (bc, bigE) awaiting PE-group emission
        for bc in range(NB):
            bsl = slice(bc * BCH, (bc + 1) * BCH)
            pb_all = pb_next
            bigE = epool.tile([128, NCHUNK, BCH], bf16, tag="bigE")
            # groups accumulated as elementwise adds instead of PE matmuls:
            # always g3 (DVE); on the last chunk also g0/g1 (DVE) and g2
            # (Pool) so the kernel tail has no serial PE emission phase
            last = bc == NB - 1
            last_chunk_copy[0] = last
            adds = ({0: nc.vector, 1: nc.gpsimd, 2: nc.gpsimd, 3: nc.vector}
                    if last else {3: nc.vector})
            accs = {g: sm.tile([128, BCH], bf16, tag=f"acc{g}",
                               name=f"acc{g}_{bc}") for g in adds}
            j0 = 0
            next_c = 0

            def process_c(c):
                nonlocal pb_next
                # weighted d-sum -> S duplicated over both 64-partition halves
                sps = ps.tile([128, BCH], f32, tag="s")
                for dj in range(4):
                    nc.tensor.matmul(
                        sps, lhsT=eye64s, rhs=bigE[:, c * 4 + dj, :],
                        start=(dj == 0), stop=(dj == 3),
                    )
                sinv = sm.tile([128, BCH], f32, tag="sinv")
                nc.vector.reciprocal_approx_fast(out=sinv, in_=sps)
                qd = sm.tile([128, BCH], bf16, tag="qd")
                nc.gpsimd.tensor_tensor(out=qd, in0=sinv, in1=pb_all[:, c, :],
                                        op=MUL)
                # Eq = E * qd in place over all 4 slices of c in one op
                qd_b = bass.AP(tensor=qd.tensor, offset=qd.offset,
                               ap=[list(qd.ap[0]), [0, 4], [1, BCH]])
                eng = nc.vector if (c % 2 == 0) != last else nc.gpsimd
                eng.tensor_tensor(
                    out=bigE[:, c * 4:(c + 1) * 4, :],
                    in0=bigE[:, c * 4:(c + 1) * 4, :],
                    in1=qd_b, op=MUL)
                # add-accumulated output groups: incremental, no PSUM
                for g, eng2 in adds.items():
                    if c == 1:
                        eng2.tensor_tensor(out=accs[g], in0=bigE[:, 0 * 4 + g, :],
                                           in1=bigE[:, 1 * 4 + g, :], op=ADD)
                    elif 2 <= c < C - 1:
                        eng2.tensor_tensor(out=accs[g], in0=accs[g],
                                           in1=bigE[:, c * 4 + g, :], op=ADD)
                    elif c == C - 1:
                        osbg = sm.tile([128, BCH], f32, tag=f"osbg{g}")
                        eng2.tensor_tensor(out=osbg, in0=accs[g],
                                           in1=bigE[:, c * 4 + g, :], op=ADD)
                        nc.sync.dma_start(
                            out=out[g * 128:(g + 1) * 128, bsl], in_=osbg)
                # interleave previous chunk's output phase to keep PE dense
                if prev is not None and c in EMIT_AT:
                    emit_out_group_pe(prev[1], prev[0], EMIT_AT[c])
                # prefetch next b-chunk's p broadcast mid-chunk
                if c == 3 and bc + 1 < NB:
                    pb_next = load_pb(bc + 1)

            for gsz in PSUM_J:
                lg = plg.tile([128, 2, BCH], f32, tag="lg")
                for t in range(gsz):
                    j = j0 + t
                    nc.tensor.matmul(
                        lg[:, t, :],
                        lhsT=W8s[:, :, j * 128:(j + 1) * 128],
                        rhs=x8s[:, :, bsl],
                        start=True, stop=True,
                        perf_mode=DR,
                    )
                nc.scalar.activation(
                    out=bigE[:, j0:j0 + gsz, :], in_=lg[:, 0:gsz, :], func=Exp,
                    scale=1.0 / (FP8_SCALE * FP8_SCALE))
                j0 += gsz
                while next_c < C and next_c * 4 + 3 < j0:
                    process_c(next_c)
                    next_c += 1
            prev = (bc, bigE)
    nc.compile()
    return nc


def _host_prep(x, p, W, b):
    W2 = W.reshape(F, K)
    bflat = b.reshape(K).astype(np.float32)
    # fp8 DoubleRow packing: [65, 2, *]; f split into two 64-deep k-tiles,
    # 65th row carries (1, bias) so the matmul adds b before exp
    S2 = FP8_SCALE * FP8_SCALE
    x8_all = np.zeros((65, 2, B_FULL), dtype=F8)
    xT = (x.T * FP8_SCALE).astype(F8)                      # (128, B)
    x8_all[:64, 0] = xT[:64]
    x8_all[:64, 1] = xT[64:]
    x8_all[64, 0] = np.float32(1.0)
    W8v = np.zeros((65, 2, K), dtype=F8)
    W8q = (W2 * FP8_SCALE).astype(F8)                      # (128, K)
    W8v[:64, 0] = W8q[:64]
    W8v[:64, 1] = W8q[64:]
    W8v[64, 0] = (bflat * S2).astype(F8)
    # pack to match the 8-piece on-chip loads: piece i holds [t=0 | t=1] for
    # k-block i*512:(i+1)*512 (dest tile iterates (t, k) per piece)
    W8 = np.ascontiguousarray(
        W8v.reshape(65, 2, 8, 512).transpose(0, 2, 1, 3)
    ).reshape(65, 2 * K)
    eye64 = ((np.arange(128)[:, None] % 64) == (np.arange(128)[None, :] % 64))
    eyes = np.zeros((128, 256), dtype=BF)
    eyes[:, 0:128] = eye64.astype(BF)
    eyes[:, 128:256] = np.eye(128, dtype=np.float32).astype(BF)
    pT_all = np.ascontiguousarray(p.T.astype(BF))          # (8, B)
    in_maps = []
    for ci in range(NCORES):
        sl = slice(ci * BS, (ci + 1) * BS)
        # piece-major x8: piece i holds [t=0 | t=1] for b-block i*512:(i+1)*512
        x8c = np.ascontiguousarray(
            x8_all[:, :, sl].reshape(65, 2, NB, BCH).transpose(0, 2, 1, 3)
        ).reshape(65, 2 * BS)
        in_maps.append({
            "x8": x8c,
            "W8": W8,
            "pT": np.ascontiguousarray(pT_all[:, sl]),
            "eyes": eyes,
        })
    return in_maps


def kernel(x, p, W, b):
    from concourse.bass_utils import run_bass_kernel_spmd

    if "nc" not in _CACHE:
        _CACHE["nc"] = _build_nc()
    nc = _CACHE["nc"]
    in_maps = _host_prep(np.asarray(x), np.asarray(p), np.asarray(W), np.asarray(b))
    res = run_bass_kernel_spmd(nc, in_maps, list(range(NCORES)))
    outs = [np.asarray(res.results[i]["out"]) for i in range(NCORES)]  # (C*R, BS)
    full = np.concatenate(outs, axis=1)              # (C*R, B)
    full = np.ascontiguousarray(full.T)              # (B, C*R)
    return full.reshape(B_FULL, C, R).astype(np.float32)


# revision 24
# speedup vs baseline: 1.0491x; 1.0169x over previous
"""Trainium2 Bass kernel for nn_CrowdsClassificationCModel.

Computes, for B x (C,C,R) annotator confusion tensors:
    logits = einsum('bf,fkr->bkr', x, W).reshape(B,C,C,R) + b
    M      = softmax(logits, axis=2)           # over predicted-class d
    out    = einsum('bc,bcdr->bdr', p, M)      # (B, C, R)

Sharding: pure data-parallel over B across 8 NeuronCores; W/b replicated.

Per-core dataflow (Bs = 2048 batch rows; k = c*512 + d*64 + r, 32 chunks of 128):
  - PE:  logits chunks (128k x 512b) via fp8e4m3 DoubleRow matmuls.
         The f=128 contraction is packed as 2 k-tiles of 64 partitions plus
         a 65th constant-one row that carries the bias b (so softmax bias
         needs no separate add anywhere). x, W scaled by 16 on host; the
         exp activation descales by 1/256.
  - ACT: E = exp(logits/256)  PSUM->SBUF bf16, 1536-wide instructions,
         into one big per-chunk E tile (128 x 32 x 512)
  - PE:  S_dup (128 x 512b) = sum_d E via exact 0/1 eye64 masked matmuls
  - DVE: sinv ~= 1/S;  Pool: qd = sinv * p_bcast
  - DVE/Pool: Eq = E * qd in place (bf16 TT, one 4-slice op per c, engines
         split by c parity)
  - PE:  c-sum for output groups 0..2: psum += I.T @ Eq (exact identity),
         interleaved one group per prev chunk c-step; DVE copies PSUM->SBUF
  - DVE: c-sum for output group 3 as incremental bf16 adds (no PSUM)
  - DMA out from SBUF (k-major; host transposes after gather)
"""

import numpy as np
import ml_dtypes

BF = ml_dtypes.bfloat16
F8 = ml_dtypes.float8_e4m3
NCORES = 8
B_FULL = 16384
BS = B_FULL // NCORES   # 2048 rows per core
F = 128
C = 8
R = 64
K = C * C * R           # 4096
NCHUNK = K // 128       # 32 k-chunks
NB = 4                  # b-chunks of 512 per core
BCH = BS // NB          # 512
FP8_SCALE = 16.0        # x and W each scaled by 16 -> exp descales by 1/256
PSUM_J = (2,) * 16      # k-chunks per logits psum tile

_CACHE = {}


def _build_nc():
    import concourse.bass as bass
    import concourse.bacc as bacc
    import concourse.tile as tile
    from concourse import mybir
    from contextlib import ExitStack

    f32 = mybir.dt.float32
    bf16 = mybir.dt.bfloat16
    fp8 = mybir.dt.float8e4
    Exp = mybir.ActivationFunctionType.Exp
    MUL = mybir.AluOpType.mult
    ADD = mybir.AluOpType.add
    DR = mybir.MatmulPerfMode.DoubleRow

    nc = bacc.Bacc()
    x8 = nc.declare_dram_parameter("x8", [65, 2 * BS], fp8, isOutput=False)
    W8 = nc.declare_dram_parameter("W8", [65, 2 * K], fp8, isOutput=False)
    pT = nc.declare_dram_parameter("pT", [C, BS], bf16, isOutput=False)
    eyes = nc.declare_dram_parameter("eyes", [128, 256], bf16, isOutput=False)
    # k-major output: row k' = d*64+r, col b; host transposes after gather
    out = nc.declare_dram_parameter("out", [C * R, BS], f32, isOutput=True)

    with ExitStack() as ctx:
        tc = ctx.enter_context(tile.TileContext(nc))
        const = ctx.enter_context(tc.tile_pool(name="const", bufs=1))
        epool = ctx.enter_context(tc.tile_pool(name="e", bufs=2))
        sm = ctx.enter_context(tc.tile_pool(name="sm", bufs=3))
        pbp = ctx.enter_context(tc.tile_pool(name="pbp", bufs=2))
        plg = ctx.enter_context(tc.tile_pool(name="plg", bufs=2, space="PSUM"))
        ps = ctx.enter_context(tc.tile_pool(name="ps", bufs=2, space="PSUM"))
        pot = ctx.enter_context(tc.tile_pool(name="pot", bufs=2, space="PSUM"))

        # const loads: W8 pieces first on SP (PE needs chunk 0 first), x8 on
        # the Pool queue, eye/identity follow on SP before first use
        W8s = const.tile([65, 2, K], fp8)
        x8s = const.tile([65, 2, BS], fp8)
        nc.sync.dma_start(out=W8s[:, :, 0:512], in_=W8[:, 0:1024])
        for i in range(NB):
            nc.gpsimd.dma_start(out=x8s[:, :, i * BCH:(i + 1) * BCH],
                                in_=x8[:, i * 2 * BCH:(i + 1) * 2 * BCH])
        eye64s = const.tile([128, 128], bf16)   # eye64 dup mask (S-sum)
        nc.sync.dma_start(out=eye64s, in_=eyes[:, 0:128])
        id128s = const.tile([128, 128], bf16)   # identity (out c-sum)
        nc.sync.dma_start(out=id128s, in_=eyes[:, 128:256])
        for i in range(1, 4):
            nc.sync.dma_start(out=W8s[:, :, i * 512:(i + 1) * 512],
                              in_=W8[:, i * 1024:(i + 1) * 1024])
        # warm the Exp activation table while DMAs stream in
        warm = const.tile([128, 1], f32)
        nc.gpsimd.memset(warm, 0.0)
        nc.scalar.activation(out=warm, in_=warm, func=Exp)

        def emit_out_group_pe(bigE, bc, g):
            """c-sum for output k-chunk g (0..2) of batch-chunk bc."""
            pog = pot.tile([128, BCH], f32, tag="ot")
            for c in range(C):
                nc.tensor.matmul(
                    pog, lhsT=id128s, rhs=bigE[:, c * 4 + g, :],
                    start=(c == 0), stop=(c == C - 1),
                )
            osb = sm.tile([128, BCH], f32, tag="osb")
            if last_chunk_copy[0]:
                nc.scalar.copy(out=osb, in_=pog)
            else:
                nc.vector.tensor_copy(out=osb, in_=pog)
            nc.sync.dma_start(
                out=out[g * 128:(g + 1) * 128, bc * BCH:(bc + 1) * BCH],
                in_=osb,
            )

        def load_pb(bc, split=False):
            # batched broadcast load of p for all 8 classes of b-chunk bc:
            # pb_all[pp, c*512 + i] = p[bc*512 + i, c]  (same for all pp)
            t = pbp.tile([128, C, BCH], bf16, tag="pb")
            pTh = pT.handle if hasattr(pT, "handle") else pT
            # split=True loads per-c pieces so c=0's slice lands early
            for c0, cn in ([(0, 1), (1, 1), (2, 2), (4, 4)] if split
                           else [(0, C)]):
                nc.sync.dma_start(
                    out=t[:, c0:c0 + cn, :],
                    in_=bass.AP(tensor=pTh, offset=bc * BCH + c0 * BS,
                                ap=[[0, 128], [BS, cn], [1, BCH]]),
                )
            return t

        EMIT_AT = {1: 0, 4: 1, 6: 2}   # prev-chunk PE emissions: c-step -> g
        last_chunk_copy = [False]
        pb_next = load_pb(0, split=True)
        for i in range(4, 8):
            nc.sync.dma_start(out=W8s[:, :, i * 512:(i + 1) * 512],
                              in_=W8[:, i * 1024:(i + 1) * 1024])
        prev = None  # (bc, bigE) awaiting PE-group emission
        for bc in range(NB):
            bsl = slice(bc * BCH, (bc + 1) * BCH)
            pb_all = pb_next
            bigE = epool.tile([128, NCHUNK, BCH], bf16, tag="bigE")
            # groups accumulated as elementwise adds instead of PE matmuls:
            # always g3 (DVE); on the last chunk also g0/g1 (DVE) and g2
            # (Pool) so the kernel tail has no serial PE emission phase
            last = bc == NB - 1
            last_chunk_copy[0] = last
            adds = ({0: nc.vector, 1: nc.gpsimd, 2: nc.gpsimd, 3: nc.vector}
                    if last else {3: nc.vector})
            accs = {g: sm.tile([128, BCH], bf16, tag=f"acc{g}",
                               name=f"acc{g}_{bc}") for g in adds}
            j0 = 0
            next_c = 0

            def process_c(c):
                nonlocal pb_next
                # weighted d-sum -> S duplicated over both 64-partition halves
                sps = ps.tile([128, BCH], f32, tag="s")
                for dj in range(4):
                    nc.tensor.matmul(
                        sps, lhsT=eye64s, rhs=bigE[:, c * 4 + dj, :],
                        start=(dj == 0), stop=(dj == 3),
                    )
                sinv = sm.tile([128, BCH], f32, tag="sinv")
                nc.vector.reciprocal_approx_fast(out=sinv, in_=sps)
                qd = sm.tile([128, BCH], bf16, tag="qd")
                tail_c = last and c == C - 1
                qd_eng = nc.vector if tail_c else nc.gpsimd
                qd_eng.tensor_tensor(out=qd, in0=sinv, in1=pb_all[:, c, :],
                                     op=MUL)
                # Eq = E * qd in place over all 4 slices of c in one op
                qd_b = bass.AP(tensor=qd.tensor, offset=qd.offset,
                               ap=[list(qd.ap[0]), [0, 2], [1, BCH]])
                if tail_c:
                    # split halves across DVE+Pool so the final adds can
                    # start as soon as their slice is scaled
                    for h, eng in ((0, nc.vector), (1, nc.gpsimd)):
                        eng.tensor_tensor(
                            out=bigE[:, c * 4 + 2 * h:c * 4 + 2 * h + 2, :],
                            in0=bigE[:, c * 4 + 2 * h:c * 4 + 2 * h + 2, :],
                            in1=qd_b, op=MUL)
                else:
                    qd_b4 = bass.AP(tensor=qd.tensor, offset=qd.offset,
                                    ap=[list(qd.ap[0]), [0, 4], [1, BCH]])
                    eng = nc.vector if (c % 2 == 0) != last else nc.gpsimd
                    eng.tensor_tensor(
                        out=bigE[:, c * 4:(c + 1) * 4, :],
                        in0=bigE[:, c * 4:(c + 1) * 4, :],
                        in1=qd_b4, op=MUL)
                # add-accumulated output groups: incremental, no PSUM
                for g, eng2 in adds.items():
                    if c == 1:
                        eng2.tensor_tensor(out=accs[g], in0=bigE[:, 0 * 4 + g, :],
                                           in1=bigE[:, 1 * 4 + g, :], op=ADD)
                    elif 2 <= c < C - 1:
                        eng2.tensor_tensor(out=accs[g], in0=accs[g],
                                           in1=bigE[:, c * 4 + g, :], op=ADD)
                    elif c == C - 1:
                        osbg = sm.tile([128, BCH], f32, tag=f"osbg{g}")
                        eng2.tensor_tensor(out=osbg, in0=accs[g],
                                           in1=bigE[:, c * 4 + g, :], op=ADD)
                        dma_eng = {0: nc.sync, 1: nc.gpsimd,
                                   2: nc.gpsimd, 3: nc.sync}[g]
                        dma_eng.dma_start(
                            out=out[g * 128:(g + 1) * 128, bsl], in_=osbg)
                # interleave previous chunk's output phase to keep PE dense
                if prev is not None and c in EMIT_AT:
                    emit_out_group_pe(prev[1], prev[0], EMIT_AT[c])
                # prefetch next b-chunk's p broadcast mid-chunk
                if c == 3 and bc + 1 < NB:
                    pb_next = load_pb(bc + 1)

            for gsz in PSUM_J:
                lg = plg.tile([128, 2, BCH], f32, tag="lg")
                for t in range(gsz):
                    j = j0 + t
                    nc.tensor.matmul(
                        lg[:, t, :],
                        lhsT=W8s[:, :, j * 128:(j + 1) * 128],
                        rhs=x8s[:, :, bsl],
                        start=True, stop=True,
                        perf_mode=DR,
                    )
                nc.scalar.activation(
                    out=bigE[:, j0:j0 + gsz, :], in_=lg[:, 0:gsz, :], func=Exp,
                    scale=1.0 / (FP8_SCALE * FP8_SCALE))
                j0 += gsz
                while next_c < C and next_c * 4 + 3 < j0:
                    process_c(next_c)
                    next_c += 1
            prev = (bc, bigE)
    nc.compile()
    return nc


def _host_prep(x, p, W, b):
    W2 = W.reshape(F, K)
    bflat = b.reshape(K).astype(np.float32)
    # fp8 DoubleRow packing: [65, 2, *]; f split into two 64-deep k-tiles,
    # 65th row carries (1, bias) so the matmul adds b before exp
    S2 = FP8_SCALE * FP8_SCALE
    x8_all = np.zeros((65, 2, B_FULL), dtype=F8)
    xT = (x.T * FP8_SCALE).astype(F8)                      # (128, B)
    x8_all[:64, 0] = xT[:64]
    x8_all[:64, 1] = xT[64:]
    x8_all[64, 0] = np.float32(1.0)
    W8v = np.zeros((65, 2, K), dtype=F8)
    W8q = (W2 * FP8_SCALE).astype(F8)                      # (128, K)
    W8v[:64, 0] = W8q[:64]
    W8v[:64, 1] = W8q[64:]
    W8v[64, 0] = (bflat * S2).astype(F8)
    # pack to match the 8-piece on-chip loads: piece i holds [t=0 | t=1] for
    # k-block i*512:(i+1)*512 (dest tile iterates (t, k) per piece)
    W8 = np.ascontiguousarray(
        W8v.reshape(65, 2, 8, 512).transpose(0, 2, 1, 3)
    ).reshape(65, 2 * K)
    eye64 = ((np.arange(128)[:, None] % 64) == (np.arange(128)[None, :] % 64))
    eyes = np.zeros((128, 256), dtype=BF)
    eyes[:, 0:128] = eye64.astype(BF)
    eyes[:, 128:256] = np.eye(128, dtype=np.float32).astype(BF)
    pT_all = np.ascontiguousarray(p.T.astype(BF))          # (8, B)
    in_maps = []
    for ci in range(NCORES):
        sl = slice(ci * BS, (ci + 1) * BS)
        # piece-major x8: piece i holds [t=0 | t=1] for b-block i*512:(i+1)*512
        x8c = np.ascontiguousarray(
            x8_all[:, :, sl].reshape(65, 2, NB, BCH).transpose(0, 2, 1, 3)
        ).reshape(65, 2 * BS)
        in_maps.append({
            "x8": x8c,
            "W8": W8,
            "pT": np.ascontiguousarray(pT_all[:, sl]),
            "eyes": eyes,
        })
    return in_maps


def kernel(x, p, W, b):
    from concourse.bass_utils import run_bass_kernel_spmd

    if "nc" not in _CACHE:
        _CACHE["nc"] = _build_nc()
    nc = _CACHE["nc"]
    in_maps = _host_prep(np.asarray(x), np.asarray(p), np.asarray(W), np.asarray(b))
    res = run_bass_kernel_spmd(nc, in_maps, list(range(NCORES)))
    outs = [np.asarray(res.results[i]["out"]) for i in range(NCORES)]  # (C*R, BS)
    full = np.concatenate(outs, axis=1)              # (C*R, B)
    full = np.ascontiguousarray(full.T)              # (B, C*R)
    return full.reshape(B_FULL, C, R).astype(np.float32)


# revision 36
# speedup vs baseline: 1.0587x; 1.0091x over previous
"""Trainium2 Bass kernel for nn_CrowdsClassificationCModel.

Computes, for B x (C,C,R) annotator confusion tensors:
    logits = einsum('bf,fkr->bkr', x, W).reshape(B,C,C,R) + b
    M      = softmax(logits, axis=2)           # over predicted-class d
    out    = einsum('bc,bcdr->bdr', p, M)      # (B, C, R)

Sharding: pure data-parallel over B across 8 NeuronCores; W/b replicated.

Per-core dataflow (Bs = 2048 batch rows; k = c*512 + d*64 + r, 32 chunks of 128):
  - PE:  logits chunks (128k x 512b) via fp8e4m3 DoubleRow matmuls.
         The f=128 contraction is packed as 2 k-tiles of 64 partitions plus
         a 65th constant-one row that carries the bias b (so softmax bias
         needs no separate add anywhere). x, W scaled by 16 on host; the
         exp activation descales by 1/256.
  - ACT: E = exp(logits/256)  PSUM->SBUF bf16, 1536-wide instructions,
         into one big per-chunk E tile (128 x 32 x 512)
  - PE:  S_dup (128 x 512b) = sum_d E via exact 0/1 eye64 masked matmuls
  - DVE: sinv ~= 1/S;  Pool: qd = sinv * p_bcast
  - DVE/Pool: Eq = E * qd in place (bf16 TT, one 4-slice op per c, engines
         split by c parity)
  - PE:  c-sum for output groups 0..2: psum += I.T @ Eq (exact identity),
         interleaved one group per prev chunk c-step; DVE copies PSUM->SBUF
  - DVE: c-sum for output group 3 as incremental bf16 adds (no PSUM)
  - DMA out from SBUF (k-major; host transposes after gather)
"""

import numpy as np
import ml_dtypes

BF = ml_dtypes.bfloat16
F8 = ml_dtypes.float8_e4m3
NCORES = 8
B_FULL = 16384
BS = B_FULL // NCORES   # 2048 rows per core
F = 128
C = 8
R = 64
K = C * C * R           # 4096
NCHUNK = K // 128       # 32 k-chunks
NB = 4                  # b-chunks of 512 per core
BCH = BS // NB          # 512
FP8_SCALE = 16.0        # x and W each scaled by 16 -> exp descales by 1/256
PSUM_J = (2,) * 16      # k-chunks per logits psum tile

_CACHE = {}


def _build_nc():
    import concourse.bass as bass
    import concourse.bacc as bacc
    import concourse.tile as tile
    from concourse import mybir
    from contextlib import ExitStack

    f32 = mybir.dt.float32
    bf16 = mybir.dt.bfloat16
    fp8 = mybir.dt.float8e4
    Exp = mybir.ActivationFunctionType.Exp
    MUL = mybir.AluOpType.mult
    ADD = mybir.AluOpType.add
    DR = mybir.MatmulPerfMode.DoubleRow

    nc = bacc.Bacc()
    x8 = nc.declare_dram_parameter("x8", [65, 2 * BS], fp8, isOutput=False)
    W8 = nc.declare_dram_parameter("W8", [65, 2 * K], fp8, isOutput=False)
    pT = nc.declare_dram_parameter("pT", [C, BS], bf16, isOutput=False)
    eyes = nc.declare_dram_parameter("eyes", [128, 256], bf16, isOutput=False)
    # k-major output: row k' = d*64+r, col b; host transposes after gather
    out = nc.declare_dram_parameter("out", [C * R, BS], f32, isOutput=True)

    with ExitStack() as ctx:
        tc = ctx.enter_context(tile.TileContext(nc))
        const = ctx.enter_context(tc.tile_pool(name="const", bufs=1))
        epool = ctx.enter_context(tc.tile_pool(name="e", bufs=2))
        sm = ctx.enter_context(tc.tile_pool(name="sm", bufs=3))
        pbp = ctx.enter_context(tc.tile_pool(name="pbp", bufs=2))
        plg = ctx.enter_context(tc.tile_pool(name="plg", bufs=2, space="PSUM"))
        ps = ctx.enter_context(tc.tile_pool(name="ps", bufs=2, space="PSUM"))
        pot = ctx.enter_context(tc.tile_pool(name="pot", bufs=2, space="PSUM"))

        # const loads: W8 pieces first on SP (PE needs chunk 0 first), x8 on
        # the Pool queue, eye/identity follow on SP before first use
        W8s = const.tile([65, 2, K], fp8)
        x8s = const.tile([65, 2, BS], fp8)
        nc.sync.dma_start(out=W8s[:, :, 0:512], in_=W8[:, 0:1024])
        for i in range(NB):
            nc.gpsimd.dma_start(out=x8s[:, :, i * BCH:(i + 1) * BCH],
                                in_=x8[:, i * 2 * BCH:(i + 1) * 2 * BCH])
        eye64s = const.tile([128, 128], bf16)   # eye64 dup mask (S-sum)
        nc.sync.dma_start(out=eye64s, in_=eyes[:, 0:128])
        id128s = const.tile([128, 128], bf16)   # identity (out c-sum)
        nc.sync.dma_start(out=id128s, in_=eyes[:, 128:256])
        for i in range(1, 4):
            nc.sync.dma_start(out=W8s[:, :, i * 512:(i + 1) * 512],
                              in_=W8[:, i * 1024:(i + 1) * 1024])
        # warm the Exp activation table while DMAs stream in
        warm = const.tile([128, 1], f32)
        nc.gpsimd.memset(warm, 0.0)
        nc.scalar.activation(out=warm, in_=warm, func=Exp)

        def emit_out_group_pe(bigE, bc, g):
            """c-sum for output k-chunk g (0..2) of batch-chunk bc."""
            pog = pot.tile([128, BCH], f32, tag="ot")
            for c in range(C):
                nc.tensor.matmul(
                    pog, lhsT=id128s, rhs=bigE[:, c * 4 + g, :],
                    start=(c == 0), stop=(c == C - 1),
                )
            osb = sm.tile([128, BCH], f32, tag="osb")
            if last_chunk_copy[0]:
                nc.scalar.copy(out=osb, in_=pog)
            else:
                nc.vector.tensor_copy(out=osb, in_=pog)
            nc.sync.dma_start(
                out=out[g * 128:(g + 1) * 128, bc * BCH:(bc + 1) * BCH],
                in_=osb,
            )

        def load_pb(bc, split=False):
            # batched broadcast load of p for all 8 classes of b-chunk bc:
            # pb_all[pp, c*512 + i] = p[bc*512 + i, c]  (same for all pp)
            t = pbp.tile([128, C, BCH], bf16, tag="pb")
            pTh = pT.handle if hasattr(pT, "handle") else pT
            # split=True loads per-c pieces so c=0's slice lands early
            for c0, cn in ([(0, 1), (1, 1), (2, 2), (4, 4)] if split
                           else [(0, C)]):
                nc.sync.dma_start(
                    out=t[:, c0:c0 + cn, :],
                    in_=bass.AP(tensor=pTh, offset=bc * BCH + c0 * BS,
                                ap=[[0, 128], [BS, cn], [1, BCH]]),
                )
            return t

        EMIT_AT = {1: 0, 4: 1, 6: 2}   # prev-chunk PE emissions: c-step -> g
        last_chunk_copy = [False]
        pb_next = load_pb(0, split=True)
        for i in range(4, 8):
            nc.sync.dma_start(out=W8s[:, :, i * 512:(i + 1) * 512],
                              in_=W8[:, i * 1024:(i + 1) * 1024])
        prev = None  # (bc, bigE) awaiting PE-group emission
        for bc in range(NB):
            bsl = slice(bc * BCH, (bc + 1) * BCH)
            pb_all = pb_next
            bigE = epool.tile([128, NCHUNK, BCH], bf16, tag="bigE")
            # groups accumulated as elementwise adds instead of PE matmuls:
            # always g3 (DVE); on the last chunk also g0/g1 (DVE) and g2
            # (Pool) so the kernel tail has no serial PE emission phase
            last = bc == NB - 1
            last_chunk_copy[0] = last
            adds = ({0: nc.vector, 1: nc.gpsimd, 2: nc.gpsimd, 3: nc.vector}
                    if last else {3: nc.vector})
            accs = {g: sm.tile([128, BCH], bf16, tag=f"acc{g}",
                               name=f"acc{g}_{bc}") for g in adds}
            j0 = 0
            next_c = 0

            def process_c(c):
                nonlocal pb_next
                # weighted d-sum -> S duplicated over both 64-partition halves
                sps = ps.tile([128, BCH], f32, tag="s")
                for dj in range(4):
                    nc.tensor.matmul(
                        sps, lhsT=eye64s, rhs=bigE[:, c * 4 + dj, :],
                        start=(dj == 0), stop=(dj == 3),
                    )
                sinv = sm.tile([128, BCH], f32, tag="sinv")
                tail_c = last and c >= C - 3
                from contextlib import nullcontext
                prio = tc.high_priority() if tail_c else nullcontext()
                with prio:
                    nc.vector.reciprocal_approx_fast(out=sinv, in_=sps)
                qd = sm.tile([128, BCH], bf16, tag="qd")
                qd_eng = nc.vector if tail_c else nc.gpsimd
                with (tc.high_priority() if tail_c else nullcontext()):
                    qd_eng.tensor_tensor(out=qd, in0=sinv, in1=pb_all[:, c, :],
                                         op=MUL)
                # Eq = E * qd in place over all 4 slices of c in one op
                qd_b = bass.AP(tensor=qd.tensor, offset=qd.offset,
                               ap=[list(qd.ap[0]), [0, 2], [1, BCH]])
                if tail_c:
                    # split halves across DVE+Pool so the final adds can
                    # start as soon as their slice is scaled
                    with tc.high_priority():
                        for h, eng in ((0, nc.vector), (1, nc.gpsimd)):
                            eng.tensor_tensor(
                                out=bigE[:, c * 4 + 2 * h:c * 4 + 2 * h + 2, :],
                                in0=bigE[:, c * 4 + 2 * h:c * 4 + 2 * h + 2, :],
                                in1=qd_b, op=MUL)
                else:
                    qd_b4 = bass.AP(tensor=qd.tensor, offset=qd.offset,
                                    ap=[list(qd.ap[0]), [0, 4], [1, BCH]])
                    eng = nc.vector if (c % 2 == 0) != last else nc.gpsimd
                    eng.tensor_tensor(
                        out=bigE[:, c * 4:(c + 1) * 4, :],
                        in0=bigE[:, c * 4:(c + 1) * 4, :],
                        in1=qd_b4, op=MUL)
                # add-accumulated output groups: incremental, no PSUM
                for g, eng2 in adds.items():
                    if c == 1:
                        eng2.tensor_tensor(out=accs[g], in0=bigE[:, 0 * 4 + g, :],
                                           in1=bigE[:, 1 * 4 + g, :], op=ADD)
                    elif 2 <= c < C - 1:
                        eng2.tensor_tensor(out=accs[g], in0=accs[g],
                                           in1=bigE[:, c * 4 + g, :], op=ADD)
                    elif c == C - 1:
                        osbg = sm.tile([128, BCH], f32, tag=f"osbg{g}")
                        with tc.high_priority():
                            eng2.tensor_tensor(out=osbg, in0=accs[g],
                                               in1=bigE[:, c * 4 + g, :], op=ADD)
                            dma_eng = {0: nc.sync, 1: nc.gpsimd,
                                       2: nc.gpsimd, 3: nc.sync}[g]
                            dma_eng.dma_start(
                                out=out[g * 128:(g + 1) * 128, bsl], in_=osbg)
                # interleave previous chunk's output phase to keep PE dense
                if prev is not None and c in EMIT_AT:
                    emit_out_group_pe(prev[1], prev[0], EMIT_AT[c])
                # prefetch next b-chunk's p broadcast mid-chunk
                if c == 3 and bc + 1 < NB:
                    pb_next = load_pb(bc + 1)

            for gsz in PSUM_J:
                lg = plg.tile([128, 2, BCH], f32, tag="lg")
                for t in range(gsz):
                    j = j0 + t
                    nc.tensor.matmul(
                        lg[:, t, :],
                        lhsT=W8s[:, :, j * 128:(j + 1) * 128],
                        rhs=x8s[:, :, bsl],
                        start=True, stop=True,
                        perf_mode=DR,
                    )
                nc.scalar.activation(
                    out=bigE[:, j0:j0 + gsz, :], in_=lg[:, 0:gsz, :], func=Exp,
                    scale=1.0 / (FP8_SCALE * FP8_SCALE))
                j0 += gsz
                while next_c < C and next_c * 4 + 3 < j0:
                    process_c(next_c)
                    next_c += 1
            prev = (bc, bigE)
    nc.compile()
    return nc


def _host_prep(x, p, W, b):
    W2 = W.reshape(F, K)
    bflat = b.reshape(K).astype(np.float32)
    # fp8 DoubleRow packing: [65, 2, *]; f split into two 64-deep k-tiles,
    # 65th row carries (1, bias) so the matmul adds b before exp
    S2 = FP8_SCALE * FP8_SCALE
    x8_all = np.zeros((65, 2, B_FULL), dtype=F8)
    xT = (x.T * FP8_SCALE).astype(F8)                      # (128, B)
    x8_all[:64, 0] = xT[:64]
    x8_all[:64, 1] = xT[64:]
    x8_all[64, 0] = np.float32(1.0)
    W8v = np.zeros((65, 2, K), dtype=F8)
    W8q = (W2 * FP8_SCALE).astype(F8)                      # (128, K)
    W8v[:64, 0] = W8q[:64]
    W8v[:64, 1] = W8q[64:]
    W8v[64, 0] = (bflat * S2).astype(F8)
    # pack to match the 8-piece on-chip loads: piece i holds [t=0 | t=1] for
    # k-block i*512:(i+1)*512 (dest tile iterates (t, k) per piece)
    W8 = np.ascontiguousarray(
        W8v.reshape(65, 2, 8, 512).transpose(0, 2, 1, 3)
    ).reshape(65, 2 * K)
    eye64 = ((np.arange(128)[:, None] % 64) == (np.arange(128)[None, :] % 64))
    eyes = np.zeros((128, 256), dtype=BF)
    eyes[:, 0:128] = eye64.astype(BF)
    eyes[:, 128:256] = np.eye(128, dtype=np.float32).astype(BF)
    pT_all = np.ascontiguousarray(p.T.astype(BF))          # (8, B)
    in_maps = []
    for ci in range(NCORES):
        sl = slice(ci * BS, (ci + 1) * BS)
        # piece-major x8: piece i holds [t=0 | t=1] for b-block i*512:(i+1)*512
        x8c = np.ascontiguousarray(
            x8_all[:, :, sl].reshape(65, 2, NB, BCH).transpose(0, 2, 1, 3)
        ).reshape(65, 2 * BS)
        in_maps.append({
            "x8": x8c,
            "W8": W8,
            "pT": np.ascontiguousarray(pT_all[:, sl]),
            "eyes": eyes,
        })
    return in_maps


def kernel(x, p, W, b):
    from concourse.bass_utils import run_bass_kernel_spmd

    if "nc" not in _CACHE:
        _CACHE["nc"] = _build_nc()
    nc = _CACHE["nc"]
    in_maps = _host_prep(np.asarray(x), np.asarray(p), np.asarray(W), np.asarray(b))
    res = run_bass_kernel_spmd(nc, in_maps, list(range(NCORES)))
    outs = [np.asarray(res.results[i]["out"]) for i in range(NCORES)]  # (C*R, BS)
    full = np.concatenate(outs, axis=1)              # (C*R, B)
    full = np.ascontiguousarray(full.T)              # (B, C*R)
    return full.reshape(B_FULL, C, R).astype(np.float32)


# revision 46
# speedup vs baseline: 1.0618x; 1.0029x over previous
"""Trainium2 Bass kernel for nn_CrowdsClassificationCModel.

Computes, for B x (C,C,R) annotator confusion tensors:
    logits = einsum('bf,fkr->bkr', x, W).reshape(B,C,C,R) + b
    M      = softmax(logits, axis=2)           # over predicted-class d
    out    = einsum('bc,bcdr->bdr', p, M)      # (B, C, R)

Sharding: pure data-parallel over B across 8 NeuronCores; W/b replicated.

Per-core dataflow (Bs = 2048 batch rows; k = c*512 + d*64 + r, 32 chunks of 128):
  - PE:  logits chunks (128k x 512b) via fp8e4m3 DoubleRow matmuls.
         The f=128 contraction is packed as 2 k-tiles of 64 partitions plus
         a 65th constant-one row that carries the bias b (so softmax bias
         needs no separate add anywhere). x, W scaled by 16 on host; the
         exp activation descales by 1/256.
  - ACT: E = exp(logits/256)  PSUM->SBUF bf16, 1024-wide instructions,
         into one big per-chunk E tile (128 x 32 x 512)
  - PE:  S_dup (128 x 512b) = sum_d E via exact 0/1 eye64 masked matmuls
  - DVE: sinv ~= 1/S;  Pool: qd = sinv * p_bcast
  - DVE/Pool: Eq = E * qd in place (bf16 TT, one 4-slice op per c, engines
         split by c parity)
  - PE:  c-sum for output groups 0..2: psum += I.T @ Eq (exact identity),
         interleaved one group per prev chunk c-step; DVE copies PSUM->SBUF
  - DVE: c-sum for output group 3 as incremental bf16 adds (no PSUM)
  - DMA out from SBUF (k-major; host transposes after gather)
"""

import numpy as np
import ml_dtypes

BF = ml_dtypes.bfloat16
F8 = ml_dtypes.float8_e4m3
NCORES = 8
B_FULL = 16384
BS = B_FULL // NCORES   # 2048 rows per core
F = 128
C = 8
R = 64
K = C * C * R           # 4096
NCHUNK = K // 128       # 32 k-chunks
NB = 4                  # b-chunks of 512 per core
BCH = BS // NB          # 512
FP8_SCALE = 16.0        # x and W each scaled by 16 -> exp descales by 1/256
PSUM_J = (2,) * 16      # k-chunks per logits psum tile

_CACHE = {}


def _build_nc():
    import concourse.bass as bass
    import concourse.bacc as bacc
    import concourse.tile as tile
    from concourse import mybir
    from contextlib import ExitStack

    f32 = mybir.dt.float32
    bf16 = mybir.dt.bfloat16
    fp8 = mybir.dt.float8e4
    Exp = mybir.ActivationFunctionType.Exp
    MUL = mybir.AluOpType.mult
    ADD = mybir.AluOpType.add
    DR = mybir.MatmulPerfMode.DoubleRow

    nc = bacc.Bacc()
    x8 = nc.declare_dram_parameter("x8", [65, 2 * BS], fp8, isOutput=False)
    W8 = nc.declare_dram_parameter("W8", [65, 2 * K], fp8, isOutput=False)
    pT = nc.declare_dram_parameter("pT", [C, BS], bf16, isOutput=False)
    eyes = nc.declare_dram_parameter("eyes", [128, 256], bf16, isOutput=False)
    # k-major output: row k' = d*64+r, col b; host transposes after gather
    out = nc.declare_dram_parameter("out", [C * R, BS], f32, isOutput=True)

    with ExitStack() as ctx:
        tc = ctx.enter_context(tile.TileContext(nc))
        const = ctx.enter_context(tc.tile_pool(name="const", bufs=1))
        epool = ctx.enter_context(tc.tile_pool(name="e", bufs=2))
        sm = ctx.enter_context(tc.tile_pool(name="sm", bufs=3))
        pbp = ctx.enter_context(tc.tile_pool(name="pbp", bufs=2))
        plg = ctx.enter_context(tc.tile_pool(name="plg", bufs=2, space="PSUM"))
        ps = ctx.enter_context(tc.tile_pool(name="ps", bufs=2, space="PSUM"))
        pot = ctx.enter_context(tc.tile_pool(name="pot", bufs=2, space="PSUM"))

        # const loads: W8 pieces first on SP (PE needs chunk 0 first), x8 on
        # the Pool queue, eye/identity follow on SP before first use
        W8s = const.tile([65, 2, K], fp8)
        x8s = const.tile([65, 2, BS], fp8)
        nc.sync.dma_start(out=W8s[:, :, 0:512], in_=W8[:, 0:1024])
        for i in range(NB):
            nc.gpsimd.dma_start(out=x8s[:, :, i * BCH:(i + 1) * BCH],
                                in_=x8[:, i * 2 * BCH:(i + 1) * 2 * BCH])
        eye64s = const.tile([128, 128], bf16)   # eye64 dup mask (S-sum)
        nc.sync.dma_start(out=eye64s, in_=eyes[:, 0:128])
        id128s = const.tile([128, 128], bf16)   # identity (out c-sum)
        nc.sync.dma_start(out=id128s, in_=eyes[:, 128:256])
        for i in range(1, 4):
            nc.sync.dma_start(out=W8s[:, :, i * 512:(i + 1) * 512],
                              in_=W8[:, i * 1024:(i + 1) * 1024])
        # warm the Exp activation table while DMAs stream in
        warm = const.tile([128, 1], f32)
        nc.gpsimd.memset(warm, 0.0)
        nc.scalar.activation(out=warm, in_=warm, func=Exp)

        def emit_out_group_pe(bigE, bc, g):
            """c-sum for output k-chunk g (0..2) of batch-chunk bc."""
            pog = pot.tile([128, BCH], f32, tag="ot")
            for c in range(C):
                nc.tensor.matmul(
                    pog, lhsT=id128s, rhs=bigE[:, c * 4 + g, :],
                    start=(c == 0), stop=(c == C - 1),
                )
            osb = sm.tile([128, BCH], f32, tag="osb")
            if last_chunk_copy[0]:
                nc.scalar.copy(out=osb, in_=pog)
            else:
                nc.vector.tensor_copy(out=osb, in_=pog)
            nc.sync.dma_start(
                out=out[g * 128:(g + 1) * 128, bc * BCH:(bc + 1) * BCH],
                in_=osb,
            )

        def load_pb(bc, split=False):
            # batched broadcast load of p for all 8 classes of b-chunk bc:
            # pb_all[pp, c*512 + i] = p[bc*512 + i, c]  (same for all pp)
            t = pbp.tile([128, C, BCH], bf16, tag="pb")
            pTh = pT.handle if hasattr(pT, "handle") else pT
            # split=True loads per-c pieces so c=0's slice lands early
            for c0, cn in ([(0, 1), (1, 1), (2, 2), (4, 4)] if split
                           else [(0, C)]):
                nc.sync.dma_start(
                    out=t[:, c0:c0 + cn, :],
                    in_=bass.AP(tensor=pTh, offset=bc * BCH + c0 * BS,
                                ap=[[0, 128], [BS, cn], [1, BCH]]),
                )
            return t

        EMIT_AT = {2: 0, 4: 1, 6: 2}   # prev-chunk PE emissions: c-step -> g
        last_chunk_copy = [False]
        pb_next = load_pb(0, split=True)
        for i in range(4, 8):
            nc.sync.dma_start(out=W8s[:, :, i * 512:(i + 1) * 512],
                              in_=W8[:, i * 1024:(i + 1) * 1024])
        prev = None  # (bc, bigE) awaiting PE-group emission
        for bc in range(NB):
            bsl = slice(bc * BCH, (bc + 1) * BCH)
            pb_all = pb_next
            bigE = epool.tile([128, NCHUNK, BCH], bf16, tag="bigE")
            # groups accumulated as elementwise adds instead of PE matmuls:
            # always g3 (DVE); on the last chunk also g0/g1 (DVE) and g2
            # (Pool) so the kernel tail has no serial PE emission phase
            last = bc == NB - 1
            last_chunk_copy[0] = last
            adds = ({0: nc.vector, 1: nc.gpsimd, 2: nc.gpsimd, 3: nc.vector}
                    if last else {3: nc.vector})
            accs = {g: sm.tile([128, BCH], bf16, tag=f"acc{g}",
                               name=f"acc{g}_{bc}") for g in adds}
            j0 = 0
            next_c = 0

            def process_c(c):
                nonlocal pb_next
                # weighted d-sum -> S duplicated over both 64-partition halves
                sps = ps.tile([128, BCH], f32, tag="s")
                for dj in range(4):
                    nc.tensor.matmul(
                        sps, lhsT=eye64s, rhs=bigE[:, c * 4 + dj, :],
                        start=(dj == 0), stop=(dj == 3),
                    )
                sinv = sm.tile([128, BCH], f32, tag="sinv")
                tail_c = last and c >= C - 3
                from contextlib import nullcontext
                prio = tc.high_priority() if tail_c else nullcontext()
                with prio:
                    nc.vector.reciprocal_approx_fast(out=sinv, in_=sps)
                qd = sm.tile([128, BCH], bf16, tag="qd")
                qd_eng = nc.vector if tail_c else nc.gpsimd
                with (tc.high_priority() if tail_c else nullcontext()):
                    qd_eng.tensor_tensor(out=qd, in0=sinv, in1=pb_all[:, c, :],
                                         op=MUL)
                # Eq = E * qd in place over all 4 slices of c in one op
                qd_b = bass.AP(tensor=qd.tensor, offset=qd.offset,
                               ap=[list(qd.ap[0]), [0, 2], [1, BCH]])
                if tail_c:
                    # split halves across DVE+Pool so the final adds can
                    # start as soon as their slice is scaled
                    with tc.high_priority():
                        for h, eng in ((0, nc.vector), (1, nc.gpsimd)):
                            eng.tensor_tensor(
                                out=bigE[:, c * 4 + 2 * h:c * 4 + 2 * h + 2, :],
                                in0=bigE[:, c * 4 + 2 * h:c * 4 + 2 * h + 2, :],
                                in1=qd_b, op=MUL)
                else:
                    qd_b4 = bass.AP(tensor=qd.tensor, offset=qd.offset,
                                    ap=[list(qd.ap[0]), [0, 4], [1, BCH]])
                    eng = nc.vector if (c % 2 == 0) != last else nc.gpsimd
                    eng.tensor_tensor(
                        out=bigE[:, c * 4:(c + 1) * 4, :],
                        in0=bigE[:, c * 4:(c + 1) * 4, :],
                        in1=qd_b4, op=MUL)
                # add-accumulated output groups: incremental, no PSUM
                for g, eng2 in adds.items():
                    if c == 1:
                        eng2.tensor_tensor(out=accs[g], in0=bigE[:, 0 * 4 + g, :],
                                           in1=bigE[:, 1 * 4 + g, :], op=ADD)
                    elif 2 <= c < C - 1:
                        eng2.tensor_tensor(out=accs[g], in0=accs[g],
                                           in1=bigE[:, c * 4 + g, :], op=ADD)
                    elif c == C - 1:
                        osbg = sm.tile([128, BCH], f32, tag=f"osbg{g}")
                        with tc.high_priority():
                            eng2.tensor_tensor(out=osbg, in0=accs[g],
                                               in1=bigE[:, c * 4 + g, :], op=ADD)
                            dma_eng = {0: nc.sync, 1: nc.gpsimd,
                                       2: nc.gpsimd, 3: nc.sync}[g]
                            dma_eng.dma_start(
                                out=out[g * 128:(g + 1) * 128, bsl], in_=osbg)
                # interleave previous chunk's output phase to keep PE dense
                if prev is not None and c in EMIT_AT:
                    emit_out_group_pe(prev[1], prev[0], EMIT_AT[c])
                # prefetch next b-chunk's p broadcast mid-chunk
                if c == 3 and bc + 1 < NB:
                    pb_next = load_pb(bc + 1)

            for gsz in PSUM_J:
                lg = plg.tile([128, 2, BCH], f32, tag="lg")
                for t in range(gsz):
                    j = j0 + t
                    nc.tensor.matmul(
                        lg[:, t, :],
                        lhsT=W8s[:, :, j * 128:(j + 1) * 128],
                        rhs=x8s[:, :, bsl],
                        start=True, stop=True,
                        perf_mode=DR,
                    )
                nc.scalar.activation(
                    out=bigE[:, j0:j0 + gsz, :], in_=lg[:, 0:gsz, :], func=Exp,
                    scale=1.0 / (FP8_SCALE * FP8_SCALE))
                j0 += gsz
                while next_c < C and next_c * 4 + 3 < j0:
                    process_c(next_c)
                    next_c += 1
            prev = (bc, bigE)
    nc.compile()
    return nc


def _host_prep(x, p, W, b):
    W2 = W.reshape(F, K)
    bflat = b.reshape(K).astype(np.float32)
    # fp8 DoubleRow packing: [65, 2, *]; f split into two 64-deep k-tiles,
    # 65th row carries (1, bias) so the matmul adds b before exp
    S2 = FP8_SCALE * FP8_SCALE
    x8_all = np.zeros((65, 2, B_FULL), dtype=F8)
    xT = (x.T * FP8_SCALE).astype(F8)                      # (128, B)
    x8_all[:64, 0] = xT[:64]
    x8_all[:64, 1] = xT[64:]
    x8_all[64, 0] = np.float32(1.0)
    W8v = np.zeros((65, 2, K), dtype=F8)
    W8q = (W2 * FP8_SCALE).astype(F8)                      # (128, K)
    W8v[:64, 0] = W8q[:64]
    W8v[:64, 1] = W8q[64:]
    W8v[64, 0] = (bflat * S2).astype(F8)
    # pack to match the 8-piece on-chip loads: piece i holds [t=0 | t=1] for
    # k-block i*512:(i+1)*512 (dest tile iterates (t, k) per piece)
    W8 = np.ascontiguousarray(
        W8v.reshape(65, 2, 8, 512).transpose(0, 2, 1, 3)
    ).reshape(65, 2 * K)
    eye64 = ((np.arange(128)[:, None] % 64) == (np.arange(128)[None, :] % 64))
    eyes = np.zeros((128, 256), dtype=BF)
    eyes[:, 0:128] = eye64.astype(BF)
    eyes[:, 128:256] = np.eye(128, dtype=np.float32).astype(BF)
    pT_all = np.ascontiguousarray(p.T.astype(BF))          # (8, B)
    in_maps = []
    for ci in range(NCORES):
        sl = slice(ci * BS, (ci + 1) * BS)
        # piece-major x8: piece i holds [t=0 | t=1] for b-block i*512:(i+1)*512
        x8c = np.ascontiguousarray(
            x8_all[:, :, sl].reshape(65, 2, NB, BCH).transpose(0, 2, 1, 3)
        ).reshape(65, 2 * BS)
        in_maps.append({
            "x8": x8c,
            "W8": W8,
            "pT": np.ascontiguousarray(pT_all[:, sl]),
            "eyes": eyes,
        })
    return in_maps


def kernel(x, p, W, b):
    from concourse.bass_utils import run_bass_kernel_spmd

    if "nc" not in _CACHE:
        _CACHE["nc"] = _build_nc()
    nc = _CACHE["nc"]
    in_maps = _host_prep(np.asarray(x), np.asarray(p), np.asarray(W), np.asarray(b))
    res = run_bass_kernel_spmd(nc, in_maps, list(range(NCORES)))
    outs = [np.asarray(res.results[i]["out"]) for i in range(NCORES)]  # (C*R, BS)
    full = np.concatenate(outs, axis=1)              # (C*R, B)
    full = np.ascontiguousarray(full.T)              # (B, C*R)
    return full.reshape(B_FULL, C, R).astype(np.float32)


# revision 51
# speedup vs baseline: 1.0647x; 1.0028x over previous
"""Trainium2 Bass kernel for nn_CrowdsClassificationCModel.

Computes, for B x (C,C,R) annotator confusion tensors:
    logits = einsum('bf,fkr->bkr', x, W).reshape(B,C,C,R) + b
    M      = softmax(logits, axis=2)           # over predicted-class d
    out    = einsum('bc,bcdr->bdr', p, M)      # (B, C, R)

Sharding: pure data-parallel over B across 8 NeuronCores; W/b replicated.

Per-core dataflow (Bs = 2048 batch rows; k = c*512 + d*64 + r, 32 chunks of 128):
  - PE:  logits chunks (128k x 512b) via fp8e4m3 DoubleRow matmuls.
         The f=128 contraction is packed as 2 k-tiles of 64 partitions plus
         a 65th constant-one row that carries the bias b (so softmax bias
         needs no separate add anywhere). x, W scaled by 16 on host; the
         exp activation descales by 1/256.
  - ACT: E = exp(logits/256)  PSUM->SBUF bf16, 1024-wide instructions,
         into one big per-chunk E tile (128 x 32 x 512)
  - PE:  S_dup (128 x 512b) = sum_d E via exact 0/1 eye64 masked matmuls
  - DVE: sinv ~= 1/S;  Pool: qd = sinv * p_bcast
  - DVE/Pool: Eq = E * qd in place (bf16 TT, one 4-slice op per c, engines
         split by c parity)
  - PE:  c-sum for output groups 0..2: psum += I.T @ Eq (exact identity),
         interleaved one group per prev chunk c-step; DVE copies PSUM->SBUF
  - DVE: c-sum for output group 3 as incremental bf16 adds (no PSUM)
  - DMA out from SBUF (k-major; host transposes after gather)
"""

import numpy as np
import ml_dtypes

BF = ml_dtypes.bfloat16
F8 = ml_dtypes.float8_e4m3
NCORES = 8
B_FULL = 16384
BS = B_FULL // NCORES   # 2048 rows per core
F = 128
C = 8
R = 64
K = C * C * R           # 4096
NCHUNK = K // 128       # 32 k-chunks
NB = 4                  # b-chunks of 512 per core
BCH = BS // NB          # 512
FP8_SCALE = 16.0        # x and W each scaled by 16 -> exp descales by 1/256
PSUM_J = (2,) * 16      # k-chunks per logits psum tile

_CACHE = {}


def _build_nc():
    import concourse.bass as bass
    import concourse.bacc as bacc
    import concourse.tile as tile
    from concourse import mybir
    from contextlib import ExitStack

    f32 = mybir.dt.float32
    bf16 = mybir.dt.bfloat16
    fp8 = mybir.dt.float8e4
    Exp = mybir.ActivationFunctionType.Exp
    MUL = mybir.AluOpType.mult
    ADD = mybir.AluOpType.add
    DR = mybir.MatmulPerfMode.DoubleRow

    nc = bacc.Bacc()
    x8 = nc.declare_dram_parameter("x8", [65, 2 * BS], fp8, isOutput=False)
    W8 = nc.declare_dram_parameter("W8", [65, 2 * K], fp8, isOutput=False)
    pT = nc.declare_dram_parameter("pT", [C, BS], bf16, isOutput=False)
    eyes = nc.declare_dram_parameter("eyes", [128, 256], bf16, isOutput=False)
    # k-major output: row k' = d*64+r, col b; host transposes after gather
    out = nc.declare_dram_parameter("out", [C * R, BS], bf16, isOutput=True)

    with ExitStack() as ctx:
        tc = ctx.enter_context(tile.TileContext(nc))
        const = ctx.enter_context(tc.tile_pool(name="const", bufs=1))
        epool = ctx.enter_context(tc.tile_pool(name="e", bufs=2))
        sm = ctx.enter_context(tc.tile_pool(name="sm", bufs=3))
        pbp = ctx.enter_context(tc.tile_pool(name="pbp", bufs=2))
        plg = ctx.enter_context(tc.tile_pool(name="plg", bufs=2, space="PSUM"))
        ps = ctx.enter_context(tc.tile_pool(name="ps", bufs=2, space="PSUM"))
        pot = ctx.enter_context(tc.tile_pool(name="pot", bufs=2, space="PSUM"))

        # const loads: W8 pieces first on SP (PE needs chunk 0 first), x8 on
        # the Pool queue, eye/identity follow on SP before first use
        W8s = const.tile([65, 2, K], fp8)
        x8s = const.tile([65, 2, BS], fp8)
        nc.sync.dma_start(out=W8s[:, :, 0:512], in_=W8[:, 0:1024])
        for i in range(NB):
            nc.gpsimd.dma_start(out=x8s[:, :, i * BCH:(i + 1) * BCH],
                                in_=x8[:, i * 2 * BCH:(i + 1) * 2 * BCH])
        eye64s = const.tile([128, 128], bf16)   # eye64 dup mask (S-sum)
        nc.sync.dma_start(out=eye64s, in_=eyes[:, 0:128])
        id128s = const.tile([128, 128], bf16)   # identity (out c-sum)
        nc.sync.dma_start(out=id128s, in_=eyes[:, 128:256])
        for i in range(1, 4):
            nc.sync.dma_start(out=W8s[:, :, i * 512:(i + 1) * 512],
                              in_=W8[:, i * 1024:(i + 1) * 1024])
        # warm the Exp activation table while DMAs stream in
        warm = const.tile([128, 1], f32)
        nc.gpsimd.memset(warm, 0.0)
        nc.scalar.activation(out=warm, in_=warm, func=Exp)

        def emit_out_group_pe(bigE, bc, g):
            """c-sum for output k-chunk g (0..2) of batch-chunk bc."""
            pog = pot.tile([128, BCH], f32, tag="ot")
            for c in range(C):
                nc.tensor.matmul(
                    pog, lhsT=id128s, rhs=bigE[:, c * 4 + g, :],
                    start=(c == 0), stop=(c == C - 1),
                )
            osb = sm.tile([128, BCH], bf16, tag="osb")
            if last_chunk_copy[0]:
                nc.scalar.copy(out=osb, in_=pog)
            else:
                nc.vector.tensor_copy(out=osb, in_=pog)
            nc.sync.dma_start(
                out=out[g * 128:(g + 1) * 128, bc * BCH:(bc + 1) * BCH],
                in_=osb,
            )

        def load_pb(bc, split=False):
            # batched broadcast load of p for all 8 classes of b-chunk bc:
            # pb_all[pp, c*512 + i] = p[bc*512 + i, c]  (same for all pp)
            t = pbp.tile([128, C, BCH], bf16, tag="pb")
            pTh = pT.handle if hasattr(pT, "handle") else pT
            # split=True loads per-c pieces so c=0's slice lands early
            for c0, cn in ([(0, 1), (1, 1), (2, 2), (4, 4)] if split
                           else [(0, C)]):
                nc.sync.dma_start(
                    out=t[:, c0:c0 + cn, :],
                    in_=bass.AP(tensor=pTh, offset=bc * BCH + c0 * BS,
                                ap=[[0, 128], [BS, cn], [1, BCH]]),
                )
            return t

        EMIT_AT = {2: 0, 4: 1, 6: 2}   # prev-chunk PE emissions: c-step -> g
        last_chunk_copy = [False]
        pb_next = load_pb(0, split=True)
        for i in range(4, 8):
            nc.sync.dma_start(out=W8s[:, :, i * 512:(i + 1) * 512],
                              in_=W8[:, i * 1024:(i + 1) * 1024])
        prev = None  # (bc, bigE) awaiting PE-group emission
        for bc in range(NB):
            bsl = slice(bc * BCH, (bc + 1) * BCH)
            pb_all = pb_next
            bigE = epool.tile([128, NCHUNK, BCH], bf16, tag="bigE")
            # groups accumulated as elementwise adds instead of PE matmuls:
            # always g3 (DVE); on the last chunk also g0/g1 (DVE) and g2
            # (Pool) so the kernel tail has no serial PE emission phase
            last = bc == NB - 1
            last_chunk_copy[0] = last
            adds = ({0: nc.vector, 1: nc.gpsimd, 2: nc.gpsimd, 3: nc.vector}
                    if last else {3: nc.vector})
            accs = {g: sm.tile([128, BCH], bf16, tag=f"acc{g}",
                               name=f"acc{g}_{bc}") for g in adds}
            j0 = 0
            next_c = 0

            def process_c(c):
                nonlocal pb_next
                # weighted d-sum -> S duplicated over both 64-partition halves
                sps = ps.tile([128, BCH], f32, tag="s")
                for dj in range(4):
                    nc.tensor.matmul(
                        sps, lhsT=eye64s, rhs=bigE[:, c * 4 + dj, :],
                        start=(dj == 0), stop=(dj == 3),
                    )
                sinv = sm.tile([128, BCH], f32, tag="sinv")
                tail_c = last and c >= C - 3
                from contextlib import nullcontext
                prio = tc.high_priority() if tail_c else nullcontext()
                with prio:
                    nc.vector.reciprocal_approx_fast(out=sinv, in_=sps)
                qd = sm.tile([128, BCH], bf16, tag="qd")
                qd_eng = nc.vector if tail_c else nc.gpsimd
                with (tc.high_priority() if tail_c else nullcontext()):
                    qd_eng.tensor_tensor(out=qd, in0=sinv, in1=pb_all[:, c, :],
                                         op=MUL)
                # Eq = E * qd in place over all 4 slices of c in one op
                qd_b = bass.AP(tensor=qd.tensor, offset=qd.offset,
                               ap=[list(qd.ap[0]), [0, 2], [1, BCH]])
                if tail_c:
                    # split halves across DVE+Pool so the final adds can
                    # start as soon as their slice is scaled
                    with tc.high_priority():
                        for h, eng in ((0, nc.vector), (1, nc.gpsimd)):
                            eng.tensor_tensor(
                                out=bigE[:, c * 4 + 2 * h:c * 4 + 2 * h + 2, :],
                                in0=bigE[:, c * 4 + 2 * h:c * 4 + 2 * h + 2, :],
                                in1=qd_b, op=MUL)
                else:
                    qd_b4 = bass.AP(tensor=qd.tensor, offset=qd.offset,
                                    ap=[list(qd.ap[0]), [0, 4], [1, BCH]])
                    eng = nc.vector if (c % 2 == 0) != last else nc.gpsimd
                    eng.tensor_tensor(
                        out=bigE[:, c * 4:(c + 1) * 4, :],
                        in0=bigE[:, c * 4:(c + 1) * 4, :],
                        in1=qd_b4, op=MUL)
                # add-accumulated output groups: incremental, no PSUM
                for g, eng2 in adds.items():
                    if c == 1:
                        eng2.tensor_tensor(out=accs[g], in0=bigE[:, 0 * 4 + g, :],
                                           in1=bigE[:, 1 * 4 + g, :], op=ADD)
                    elif 2 <= c < C - 1:
                        eng2.tensor_tensor(out=accs[g], in0=accs[g],
                                           in1=bigE[:, c * 4 + g, :], op=ADD)
                    elif c == C - 1:
                        osbg = sm.tile([128, BCH], bf16, tag=f"osbg{g}")
                        with tc.high_priority():
                            eng2.tensor_tensor(out=osbg, in0=accs[g],
                                               in1=bigE[:, c * 4 + g, :], op=ADD)
                            dma_eng = {0: nc.sync, 1: nc.gpsimd,
                                       2: nc.gpsimd, 3: nc.sync}[g]
                            dma_eng.dma_start(
                                out=out[g * 128:(g + 1) * 128, bsl], in_=osbg)
                # interleave previous chunk's output phase to keep PE dense
                if prev is not None and c in EMIT_AT:
                    emit_out_group_pe(prev[1], prev[0], EMIT_AT[c])
                # prefetch next b-chunk's p broadcast mid-chunk
                if c == 3 and bc + 1 < NB:
                    pb_next = load_pb(bc + 1)

            for gsz in PSUM_J:
                lg = plg.tile([128, 2, BCH], f32, tag="lg")
                for t in range(gsz):
                    j = j0 + t
                    nc.tensor.matmul(
                        lg[:, t, :],
                        lhsT=W8s[:, :, j * 128:(j + 1) * 128],
                        rhs=x8s[:, :, bsl],
                        start=True, stop=True,
                        perf_mode=DR,
                    )
                nc.scalar.activation(
                    out=bigE[:, j0:j0 + gsz, :], in_=lg[:, 0:gsz, :], func=Exp,
                    scale=1.0 / (FP8_SCALE * FP8_SCALE))
                j0 += gsz
                while next_c < C and next_c * 4 + 3 < j0:
                    process_c(next_c)
                    next_c += 1
            prev = (bc, bigE)
    nc.compile()
    return nc


def _host_prep(x, p, W, b):
    W2 = W.reshape(F, K)
    bflat = b.reshape(K).astype(np.float32)
    # fp8 DoubleRow packing: [65, 2, *]; f split into two 64-deep k-tiles,
    # 65th row carries (1, bias) so the matmul adds b before exp
    S2 = FP8_SCALE * FP8_SCALE
    x8_all = np.zeros((65, 2, B_FULL), dtype=F8)
    xT = (x.T * FP8_SCALE).astype(F8)                      # (128, B)
    x8_all[:64, 0] = xT[:64]
    x8_all[:64, 1] = xT[64:]
    x8_all[64, 0] = np.float32(1.0)
    W8v = np.zeros((65, 2, K), dtype=F8)
    W8q = (W2 * FP8_SCALE).astype(F8)                      # (128, K)
    W8v[:64, 0] = W8q[:64]
    W8v[:64, 1] = W8q[64:]
    W8v[64, 0] = (bflat * S2).astype(F8)
    # pack to match the 8-piece on-chip loads: piece i holds [t=0 | t=1] for
    # k-block i*512:(i+1)*512 (dest tile iterates (t, k) per piece)
    W8 = np.ascontiguousarray(
        W8v.reshape(65, 2, 8, 512).transpose(0, 2, 1, 3)
    ).reshape(65, 2 * K)
    eye64 = ((np.arange(128)[:, None] % 64) == (np.arange(128)[None, :] % 64))
    eyes = np.zeros((128, 256), dtype=BF)
    eyes[:, 0:128] = eye64.astype(BF)
    eyes[:, 128:256] = np.eye(128, dtype=np.float32).astype(BF)
    pT_all = np.ascontiguousarray(p.T.astype(BF))          # (8, B)
    in_maps = []
    for ci in range(NCORES):
        sl = slice(ci * BS, (ci + 1) * BS)
        # piece-major x8: piece i holds [t=0 | t=1] for b-block i*512:(i+1)*512
        x8c = np.ascontiguousarray(
            x8_all[:, :, sl].reshape(65, 2, NB, BCH).transpose(0, 2, 1, 3)
        ).reshape(65, 2 * BS)
        in_maps.append({
            "x8": x8c,
            "W8": W8,
            "pT": np.ascontiguousarray(pT_all[:, sl]),
            "eyes": eyes,
        })
    return in_maps


def kernel(x, p, W, b):
    from concourse.bass_utils import run_bass_kernel_spmd

    if "nc" not in _CACHE:
        _CACHE["nc"] = _build_nc()
    nc = _CACHE["nc"]
    in_maps = _host_prep(np.asarray(x), np.asarray(p), np.asarray(W), np.asarray(b))
    res = run_bass_kernel_spmd(nc, in_maps, list(range(NCORES)))
    outs = [np.asarray(res.results[i]["out"]).astype(np.float32)
            for i in range(NCORES)]                  # (C*R, BS)
    full = np.concatenate(outs, axis=1)              # (C*R, B)
    full = np.ascontiguousarray(full.T)              # (B, C*R)
    return full.reshape(B_FULL, C, R).astype(np.float32)
